# revision 1
# baseline (speedup 1.0000x reference)
"""Trainium2 Bass kernel for nn_ADVI (segment_reduce ELBO).

Math:
  elbo = const(prior - q) + sum_n LSE_c( ll[n,c] + log_pis[ks_n, c, ts_n] )
  ll[n,c] is a quadratic form in s_tilde = [s; 1]; all 32 class quadratics are
  represented EXACTLY (residual ~1e-6) as  ll[n,c] = sum_m lam[m,c] * (v_m . s_t)^2 + K_c
  with 64 shared directions v_m -> two matmuls with a square between them,
  2 spikes packed per moving column (K=22, M=128).
  The per-spike table row  T'[j,c] = log_pis[j//128, c, j%128] + K_c  is gathered
  from HBM by the SWDGE dma_gather (elem = 64 f32 = 256B) with j = ks*128+ts.

Per core: 125000 spikes, padded to 126976 = 31 chunks x 4096.
Pipeline per chunk: DMA loads -> PE mm1 (proj) -> ACT square -> PE mm2 (lam) ->
DVE add gathered rows -> ACT exp -> DVE segmented reduce -> ACT log -> accum.
All engines software-pipelined via cumulative semaphores.
"""

import base64
import sys
from contextlib import ExitStack

import numpy as np

sys.path.insert(0, "/opt/trn_rl_repo")

import ml_dtypes  # noqa: E402

BF16 = ml_dtypes.bfloat16

LOG2PI = float(np.log(2.0 * np.pi))
N_K, N_T, N_C, N_D, N_SPK = 256, 128, 32, 10, 1000000
N_CORES = 8
NLOC = N_SPK // N_CORES           # 125000
CHUNK = 4096                      # spikes per chunk
NCHUNK = 31                       # 31*4096 = 126976 >= 125000
NPAD_LOC = NCHUNK * CHUNK - NLOC  # 1976
NPAIR = NCHUNK * CHUNK // 2       # 63488
M_DIRS = 64

# 64 fitted directions (f64, 64x11), warm start for the in-kernel refinement.
_V64_B64 = """
AAAAoH/q7z8AAADAZxKMPwAAAGC3gpK/AAAAYMKUkD8AAABAftuSvwAAAMD4rJq/AAAAIPyonj8AAABgqPmTPwAAAEAhMZI/AAAA
wOuImD8AAADgfAYIvwAAAAA0dIq/AAAAAG1L8D8AAABgiy+LvwAAAOChXpI/AAAAIJoyVj8AAACAp5SfPwAAACBKWni/AAAAQP9B
Qj8AAACA7E9svwAAAEAbVoO/AAAAAMFbk78AAABgyQqWPwAAAACRppK/AAAAgNFA7z8AAADgRMSUvwAAACAXYqE/AAAA4C9omL8A
AABgzk+MvwAAAACFkaE/AAAAgBYBkD8AAABAwQmbPwAAAMDHDY4/AAAAIAfUeL8AAACgFpiZvwAAAMB6ezK/AAAAoFOO8D8AAAAA
fSGRPwAAAMDzCpo/AAAAgFvYob8AAACAdd9zvwAAAAAHsYy/AAAAYDYjdj8AAAAgq6ybvwAAAGCAyZi/AAAAwORZeL8AAAAgkaqb
PwAAAGBoKJc/AAAAADpY7z8AAAAgFQabPwAAAGCR5Za/AAAAoN5dcT8AAADA7yyiPwAAACBJGWm/AAAAgMEakD8AAAAg5MGHPwAA
ACDI6om/AAAA4Bu6mr8AAAAghwSgvwAAAEBQuJ2/AAAAAIlT8D8AAACAoh2NvwAAAMDATKe/AAAAIPH4lj8AAACAN3JRPwAAAEA+
ipG/AAAA4MOIoj8AAADAWhqLvwAAAOAoZpA/AAAAIBbdkz8AAAAAv2GXvwAAAECqzJu/AAAAgDgG8D8AAAAAlYlaPwAAAGD6cIA/
AAAAgEdPmD8AAACAFWt9vwAAAGC7wZM/AAAAIG9KlL8AAABAXB2gPwAAACCKw5Q/AAAAwNc1hb8AAADAuhebPwAAAKAdFYg/AAAA
oImF8D8AAAAgVhyRvwAAAKAOMUU/AAAAoJFBq78AAADgFmd5PwAAAACdT5Q/AAAAwBoOlL8AAADAOPyOvwAAAOAhHpo/AAAAQJLF
k78AAADgQe+VPwAAAOBWfJq/AAAAgLgi8D8AAADA2yyCPwAAAOC96Y2/AAAAoCRGob8AAACASjBtPwAAACD7GIY/AAAAoLxHdj8A
AADA9/mDPwAAAGAT/YQ/AAAAQKxiZr8AAABAolWSvwAAAECaAnu/AAAAAJx/8D8AAABAzH2cvwAAACBK9+8/AAAAYAyajb8AAAAA
X8SUPwAAAGAMKZC/AAAAoF+YlD8AAADANBCYPwAAACCmSZK/AAAAwNllj78AAAAgHyeMvwAAAGBB1Zq/AAAAwHEo8D8AAACAiuyO
PwAAAGD4fO8/AAAAwJRTiz8AAADA1XWLvwAAAICtNHG/AAAA4Nbxn78AAADAWD1/PwAAAECEtD8/AAAAQPTfRT8AAABg852GPwAA
AIDISPA/AAAAwBRfkb8AAAAgOxJzPwAAAID6zO4/AAAAoJ/pfD8AAACA2zJ+vwAAAADpEoQ/AAAAAJQ7oj8AAAAAToeTvwAAAMDi
bYi/AAAAINCVj78AAACgrofwPwAAAMBaQ4w/AAAAoDJLnj8AAABAIxN8PwAAAOBBo/A/AAAAYCNFib8AAACgHnycvwAAAEDB5Kc/
AAAAIJG/jD8AAADgvKuBPwAAAABjioW/AAAAIId67j8AAABA91GXPwAAAEBYZU4/AAAAQHxIn78AAABgvBOWvwAAAIABS/A/AAAA
gNsAmL8AAABA1syXPwAAAOBNSW6/AAAAQEUUob8AAAAggTxivwAAACBQje8/AAAAoNnyhb8AAABg1N6DPwAAAGD3bmm/AAAAYCZT
mj8AAACg1kqdPwAAACBTo+4/AAAAAE4fij8AAADAnm+gPwAAAACXX4m/AAAA4HqUir8AAACAip7wPwAAAOCFKqK/AAAAIC0Qaz8A
AACgwpSavwAAAOC8aIy/AAAAACfalj8AAACg4viTPwAAAMBAte8/AAAAwHGml78AAABg6zaNvwAAAODNapC/AAAAgGk48D8AAADg
TGqcvwAAAID7tJE/AAAAYMYOob8AAAAg7A2RvwAAAIC4fWy/AAAAgNxWib8AAABgAteRvwAAACA2X+4/AAAAAKAnoD8AAAAAwJSG
vwAAAADtFPE/AAAAAA70hb8AAABAA9CTvwAAAKAxrpQ/AAAAoPUEkD8AAADAjSKYvwAAAIBnKJU/AAAAINyOmL8AAABA3Q6gPwAA
AKAsYPA/AAAAIOqql78AAACAPYzvPwAAAKAI3Zw/AAAAAOydYb8AAADAK5yAvwAAAEAQt32/AAAAoNLdhL8AAABg8g6UvwAAAOCo
cXI/AAAAIM91iz8AAADA9oyCPwAAAAAzT+8/AAAAgFRm8D8AAAAAf3PrPwAAAOBKGeI/AAAAgIl40r8AAADAE/TgvwAAAICSDrW/
AAAAIBOK6T8AAABgUrbMPwAAAIBGiuY/AAAAAIjZyD8AAACgLbjOPwAAAMB3EvI/AAAAYAxvor8AAACgqNzuPwAAAICA9ea/AAAA
4NskwT8AAAAAddHCPwAAACANYem/AAAAQMSt4L8AAAAgwJTHPwAAAIDdOqa/AAAAIIMe1j8AAADgOZmwvwAAAIAOZeE/AAAAgJKm
wj8AAACAZgDQvwAAAEDoc7m/AAAA4PbP1b8AAADA1VfYPwAAAICHW8o/AAAAoLXd3j8AAABAYhLLPwAAAKBdIrI/AAAAwHWv878A
AABgd4HjvwAAAICvlsm/AAAAAJkizr8AAADAI+LWvwAAAKBTx9e/AAAAADWflb8AAABgFzunPwAAAMARQa4/AAAAYIPZ5z8AAACg
mI/lPwAAAAAc66q/AAAAALOHxz8AAABgJ1boPwAAAKC4iNc/AAAAQByC4T8AAABADrHlvwAAAABrcry/AAAAILZ+5b8AAADAaXDw
vwAAAEDQHve/AAAAAEMNxr8AAABgc0f7PwAAAEAQHao/AAAA4Dnj2b8AAADgnjTJvwAAAIB5gNG/AAAAANUv4T8AAADgwC+jPwAA
AIC7adM/AAAAoCda4T8AAABA/CjwPwAAAOBztNA/AAAAAOFp+z8AAADgKvjJPwAAAABcaNw/AAAAgOFDtz8AAADgFR+xvwAAAMD2
ZPC/AAAAoE8h7z8AAABgIRzqPwAAAOAFl+E/AAAA4Gp7rr8AAABgbAHrPwAAAEDb37Y/AAAAIOWg578AAAAAiDruvwAAACD4pOS/
AAAAgKwc0L8AAABgtoHhvwAAAGAxoMS/AAAA4Nh81b8AAAAgfUPhvwAAAIBxJKy/AAAAoIOc7T8AAAAgiha4PwAAAGAu+/Y/AAAA
gIUH0T8AAABA88TWvwAAAMB9V+U/AAAA4PUWu78AAACAgZC+PwAAAEBuItO/AAAAYGIUnj8AAACA3EzyPwAAAGDVDtk/AAAAYP1t
+z8AAAAA7irtPwAAAOBVd7C/AAAAQGTjyL8AAAAgUcnpPwAAAIAnvqm/AAAAIJ8vUz8AAABAgQ65vwAAACB/GHw/AAAAoBK07T8A
AAAgVaDQPwAAAID8Pvu/AAAAQItmZr8AAAAAWuvzvwAAAAAM+MK/AAAA4CjN1j8AAAAAqDHVPwAAAIB858C/AAAAQHXvyD8AAACg
eYXxvwAAACBNCrS/AAAAAFOLxT8AAABgYzSyPwAAAMAM5uU/AAAAQFuAtD8AAAAgxdbUPwAAAEDha+W/AAAAQNZyzT8AAADA7iHq
PwAAAABZG+Y/AAAAAEi/1b8AAAAgVrDYvwAAAGC8oeM/AAAAgKM3fT8AAADgjq/VPwAAAEAIYu6/AAAAoFHN0j8AAABAFK+1PwAA
AKBrLcO/AAAAgHIKlz8AAADA83ruPwAAAEBkHMc/AAAAQP+iuz8AAACgIl3lvwAAAEBIWfW/AAAAoEg24D8AAABAD5DmvwAAAICR
8Lk/AAAAADnllr8AAACgAJrSvwAAAKBcgre/AAAAgMwU6D8AAAAgkqCqPwAAAEAjKr8/AAAAQF5h1L8AAAAgcib1PwAAAMAFHuM/
AAAAQF6b5D8AAABAI+3VPwAAAIBMt74/AAAA4AdRxD8AAABAVvnmPwAAAOCJStY/AAAAoM22xz8AAACgerzlvwAAAAB3Ooq/AAAA
oFWypT8AAACAkq3DPwAAAODLEdE/AAAAAMRs0j8AAADAFMXLvwAAAEDbuu2/AAAAoONnvj8AAABAJpSOvwAAAKBz8NI/AAAAAH7g
2j8AAACAmsXhPwAAAGCb8se/AAAAYIZf3j8AAAAAWEDqPwAAAOCGC6k/AAAAwBBN0j8AAADACEDlPwAAAGBDGd+/AAAAwAfY4L8A
AACgeYLxPwAAAICzEMG/AAAAICyI5r8AAADA3rv5PwAAAGDgAtK/AAAA4LpZ4r8AAABgj7ChvwAAAOCy0ti/AAAAQGyZ5r8AAABg
QX/RPwAAAKBhTcM/AAAAQG8S478AAABACtq8PwAAAOANg+I/AAAAAO96+T8AAACArrzlPwAAAIC+5PK/AAAAQG1Azr8AAACAkbCi
PwAAAEBzXuY/AAAA4Pkdrz8AAADgRKLMvwAAAAA7T8m/AAAAQM/C178AAADgrXfJvwAAAADkWbI/AAAAwMVS4r8AAAAA3GfWvwAA
AECtbde/AAAAYDdC2T8AAAAAYo7ivwAAAODOOui/AAAA4M31xj8AAACgxD+4vwAAACBvQtW/AAAAYIa/6r8AAABgqu2RvwAAAIBl
y/I/AAAAoL8Oyb8AAABgzknfvwAAAGB0D6s/AAAAYNkwkz8AAACghJPoPwAAAEAvFtI/AAAAoEz13D8AAABAPQLbPwAAACDeLbS/
AAAA4O/p+L8AAACA6QTyvwAAAED5JcU/AAAAoP6L0z8AAABgdP9XPwAAAKARV8e/AAAAgDaj6L8AAADAeTjdvwAAAABgQNS/AAAA
AGOR4b8AAADABMjDvwAAAAAowPi/AAAAoEGt0L8AAAAgRU3LvwAAAGCA7+C/AAAAAB5H2L8AAAAg6J/evwAAAOAxENg/AAAAQIOU
vL8AAAAgmSnivwAAAEBeHrg/AAAAgOpb0r8AAADASHCsvwAAAACKJs2/AAAAgGp54j8AAAAA3b3DvwAAAMBZXEe/AAAAQFR11r8A
AACAzuKgPwAAAOCcTOc/AAAAwJVtsz8AAABg7iiqvwAAAMCVEdE/AAAAIOtXlb8AAABgNh/avwAAAEAXUOC/AAAA4FG15L8AAAAg
60eivwAAAMCEVto/AAAAYFLJlr8AAACgHCPwPwAAAKBxsOg/AAAAYKygzj8AAADAH4zGvwAAACBmbvm/AAAAADI4zz8AAABg9RXh
PwAAAKAeiNc/AAAAYP6fwT8AAABgbOfTvwAAACAkSsU/AAAAYD527r8AAABgxBLvvwAAACBxttG/AAAAgNkvxT8AAACgxQb6vwAA
AICUDNs/AAAAwIM9xz8AAADgLjXiPwAAAEDbas+/AAAAYCUQ1L8AAACghOfAvwAAAOBB07c/AAAAwNAs2D8AAADgjBqjvwAAAGDZ
kum/AAAAYP0rwb8AAACg2zXHPwAAAOD3MX2/AAAAwHcypD8AAABgbZjePwAAAEBJb78/AAAA4Ky6zj8AAABACfHaPwAAAGAOpeC/
AAAAgAnHyT8AAABA66vWvwAAAECJtLI/AAAAAJWywL8AAABA9mXzvwAAAKD9H+6/AAAAgJ0S0j8AAABA2ZvZvwAAAABlAJQ/AAAA
wKY64T8AAAAgRSvQPwAAAACo6ts/AAAAoIBK7L8AAACgUCD+vwAAAKATcrU/AAAAwB0u8L8AAAAAJCXqvwAAAIBCbt0/AAAAIKXe
4L8AAABANdC1vwAAAGBGy9g/AAAAoNoDxT8AAAAAeK7fPwAAAGAYIua/AAAAgJsM/j8AAABAOVXYvwAAAEC/6cK/AAAAQMeT5L8A
AADAzirBvwAAAGA0m6k/AAAAwDXc478AAACAd9LevwAAACB6O9g/AAAAQOey8b8AAABAsSu6PwAAAGAIPZy/AAAAYA+fuz8AAACg
wnCzvwAAACCzI+E/AAAAwG4KnL8AAACg+vnyvwAAAIBVn8s/AAAAQJP0pD8AAACAH6LhPwAAAOCREtG/AAAAgOL9xT8AAADAYi22
vwAAAADJgPI/AAAAwHsd6z8AAACAYtjhPwAAACA/P9s/AAAAoLEF+D8AAADgNBjhvwAAACCPFdq/AAAAoDMWcT8AAADgaCHjPwAA
ACAtf8G/AAAAQDKGAEAAAADge4XovwAAAEBblN+/AAAAgHQe0L8AAAAgEmDMvwAAAEAhM/O/AAAAwCKf3z8AAADAsK/cPwAAAADc
xYm/AAAAwGea4L8AAAAgsj/DPwAAAIDCbgBAAAAAAATR9L8AAACgvTnYPwAAAICb38g/AAAAYEgNyb8AAADgLpm3PwAAAODNSt2/
AAAA4Dnqy78AAABgb2rkPwAAAADV9/I/AAAAoFGq1T8AAACgs7asPwAAAIA6UKW/AAAAAIsUxD8AAADAQdLmPwAAAMDJnNG/AAAA
ICnNt78AAADABcflPwAAAGCr754/AAAAwJXC6z8AAADgH1G2vwAAAGARP/a/AAAAgA9Pvb8AAACgcTrkvwAAAEB3Y8W/AAAAgDXO
8z8AAABgajPkvwAAAKBIM+m/AAAAANPO6D8AAACAOx7APwAAAAA52r+/AAAAQGFD4b8AAADAXdzwPwAAAAAGTwDAAAAAIKQU4b8A
AABg6OTIvwAAAODaBfA/AAAAgCVb2r8AAAAAw/3hvwAAAKCqAeM/AAAAwOnRij8AAADAoCLMvwAAAKBWaea/AAAA4P+q7T8AAACA
OBUAQAAAAECR/ty/AAAAoGzW5b8AAABgYS6wvwAAAOBvJKa/AAAAwN6H0D8AAAAA56zhvwAAAMDFM9a/AAAAwBsT7r8AAAAARH/Q
vwAAAOBbjLe/AAAAwHpZkL8AAACATezjvwAAAECzEHQ/AAAAQMvO5b8AAABA7VJgvwAAAGCtpuG/AAAAIG5Byb8AAAAgLjisvwAA
AOAhb8E/AAAAIN/o4j8AAACgR33iPwAAAMDPE4i/AAAAALb/zD8AAAAgUa3UPwAAAMAJR8C/AAAAAIAZ4r8AAAAAF4u7PwAAACAa
ruI/AAAAABfXzr8AAACAzfzyvwAAAKCUeN2/AAAAgL/57L8AAABA7kv7PwAAAEB7uNi/AAAAINzb0r8AAADg637OPwAAAIAzQNs/
AAAAgD/Hp78AAABgWzXYvwAAAMDUusQ/AAAAwMHP8T8AAADA4SjcPwAAAKCMA+k/AAAAwESr/D8AAACgSeiqvwAAAMBhpt0/AAAA
IGnxzz8AAACAXmPcPwAAACD+XKC/AAAAIOwc6D8AAAAAc+rePwAAAEALMqG/AAAA4F05ub8AAADAAhPIPwAAAAAp94i/AAAAYFG5
1r8AAAAAtHSTvwAAAEBUo+G/AAAA4ErdhD8AAADg7YrkPwAAAKARbMw/AAAAoCG9vr8AAADgnoyuPwAAAEAXH7M/AAAAIM5VoD8A
AACgMD2HPw==
"""


def _decode_v():
    b = base64.b64decode("".join(_V64_B64.split()))
    return np.frombuffer(b, dtype=np.float64).reshape(M_DIRS, 11).copy()


# ----------------------------------------------------------------------------
# host-side math prep (small params only; O(table) work, no O(n_spk) compute)
# ----------------------------------------------------------------------------

def _fit_dirs(P_t):
    """Refine the 64 directions against the actual P_tilde set (warm start).
    P_t: (32, 11, 11), entry (10,10) is free. Returns V (64,11), Lam (64,32)."""
    V = _decode_v()
    mask = np.ones((11, 11)); mask[10, 10] = 0.0
    mask_f = mask.reshape(-1)
    Pf = (P_t.reshape(N_C, -1) * mask_f)          # (32,121)
    scale = np.linalg.norm(Pf)

    def lam_and_res(V):
        B = np.einsum('mi,mj->mij', V, V).reshape(M_DIRS, -1) * mask_f
        Lam, *_ = np.linalg.lstsq(B.T, Pf.T, rcond=None)
        R = B.T @ Lam - Pf.T                      # (121,32)
        return Lam, R, B

    Lam, R, B = lam_and_res(V)
    res = np.linalg.norm(R) / scale
    if res > 1e-5:
        # inputs differ from the fit-time reference: refine with Adam-ALS
        m = np.zeros_like(V); v2 = np.zeros_like(V)
        lr, b1, b2, eps = 3e-3, 0.9, 0.999, 1e-8
        best = (np.inf, V.copy(), Lam)
        for it in range(4000):
            Lam, R, B = lam_and_res(V)
            loss = float(np.sum(R * R))
            if loss < best[0]:
                best = (loss, V.copy(), Lam)
            if np.sqrt(loss) / scale < 2e-6:
                break
            # grad wrt V:  d/dv_m ||sum_c (lam_mc v v^T - P)||^2
            Rc = (R.T).reshape(N_C, 11, 11) * mask  # (32,11,11)
            G = np.einsum('mc,cij,mj->mi', Lam, Rc + Rc.transpose(0, 2, 1), V)
            m = b1 * m + (1 - b1) * G
            v2 = b2 * v2 + (1 - b2) * G * G
            mh = m / (1 - b1 ** (it + 1)); vh = v2 / (1 - b2 ** (it + 1))
            V = V - lr * mh / (np.sqrt(vh) + eps)
        loss, V, Lam = best
        res = np.sqrt(loss) / scale
    return V, Lam, res


def _host_prep(s, y, ks, ts, means, covs, b_mu, b_log_sig, beta_mu, beta_log_sig):
    f8 = np.float64
    means8, covs8 = means.astype(f8), covs.astype(f8)
    P = np.linalg.inv(covs8)
    P = 0.5 * (P + P.transpose(0, 2, 1))
    sign, logdet = np.linalg.slogdet(covs8)
    assert np.all(sign > 0)

    # P_tilde (32,11,11): s_t^T Pt s_t = -0.5 s P s + (P mu).s   ((10,10) free)
    Pt = np.zeros((N_C, 11, 11))
    Pt[:, :10, :10] = -0.5 * P
    w = np.einsum('cij,cj->ci', P, means8)
    Pt[:, :10, 10] = 0.5 * w
    Pt[:, 10, :10] = 0.5 * w

    V, Lam, res = _fit_dirs(Pt)

    # device computes D_c = s_t^T E_c s_t,  E = sum_m lam vv^T
    E = np.einsum('mc,mi,mj->cij', Lam, V, V)
    muPmu = np.einsum('ci,cij,cj->c', means8, P, means8)
    K = -E[:, 10, 10] - 0.5 * muPmu - 0.5 * logdet - 0.5 * N_D * LOG2PI

    # log_pis table (f64) + K fold -> T' (32768, 32) f32, padded to 64
    y8 = y.astype(f8)
    ll_kct = b_mu.astype(f8)[None, :, None] + beta_mu.astype(f8)[None, :, :] * y8[:, None, :]
    mx = ll_kct.max(axis=1, keepdims=True)
    lse = mx + np.log(np.exp(ll_kct - mx).sum(axis=1, keepdims=True))
    log_pis = ll_kct - lse                                  # (256,32,128)
    Tp = log_pis.transpose(0, 2, 1).reshape(N_K * N_T, N_C) + K[None, :]
    Tpad = np.zeros((N_K * N_T, 64), dtype=np.float32)
    Tpad[:, :N_C] = Tp.astype(np.float32)

    # prior - q const (f64, formulas of the reference)
    lp = -0.5 * (b_mu.astype(f8) ** 2 + LOG2PI).sum() \
         - 0.5 * (beta_mu.astype(f8) ** 2 + LOG2PI).sum()
    lq = (-0.5 * LOG2PI * b_mu.size - b_log_sig.astype(f8).sum()) + \
         (-0.5 * LOG2PI * beta_mu.size - beta_log_sig.astype(f8).sum())
    elbo_const = lp - lq

    # pad spikes: s_t = 0 (all-zero col) -> D=0; idx=0 -> row T'[0]
    r0 = Tpad[0, :N_C].astype(f8)
    m0 = r0.max()
    lse_pad = m0 + np.log(np.exp(r0 - m0).sum())

    # W stationary (22, 128) bf16
    W = np.zeros((22, 128), dtype=np.float32)
    W[0:11, 0:M_DIRS] = V.T.astype(np.float32)
    W[11:22, M_DIRS:2 * M_DIRS] = V.T.astype(np.float32)

    lam32 = np.vstack([Lam, Lam]).astype(np.float32)        # (128,32)

    # per-core spike data
    idx_all = (ks.astype(np.int64) * N_T + ts.astype(np.int64)).astype(np.int16)
    s_aug = np.concatenate([s.astype(np.float32),
                            np.ones((N_SPK, 1), np.float32)], axis=1)  # (n,11)

    sp_cores, idx_cores = [], []
    for i in range(N_CORES):
        sl = slice(i * NLOC, (i + 1) * NLOC)
        sa = np.zeros((NCHUNK * CHUNK, 11), dtype=np.float32)
        sa[:NLOC] = s_aug[sl]
        ia = np.zeros((NCHUNK * CHUNK,), dtype=np.int16)
        ia[:NLOC] = idx_all[sl]
        # pack pairs: (npair, 2, 11) -> (22, npair)
        spk = sa.reshape(NPAIR, 2, 11).transpose(1, 2, 0).reshape(22, NPAIR)
        sp_cores.append(spk.astype(BF16))
        # idx permute to gather order: chunk-local (16w,128p,2a) -> (w,a,p)
        iv = ia.reshape(NCHUNK, 16, 128, 2).transpose(0, 1, 3, 2).reshape(NCHUNK, CHUNK)
        # wrap in 16 partitions, replicate x8 -> (NCHUNK*128, 256)
        iw = iv.reshape(NCHUNK, 256, 16).transpose(0, 2, 1)       # (NCHUNK,16,256)
        iw = np.tile(iw, (1, 8, 1)).reshape(NCHUNK * 128, 256)
        idx_cores.append(np.ascontiguousarray(iw))

    return dict(Tpad=Tpad, W=W.astype(BF16), lam=lam32.astype(BF16),
                sp_cores=sp_cores, idx_cores=idx_cores,
                elbo_const=elbo_const, lse_pad=lse_pad, fit_res=res)


# ----------------------------------------------------------------------------
# device graph
# ----------------------------------------------------------------------------

_GRAPH = None


def _build_graph():
    global _GRAPH
    if _GRAPH is not None:
        return _GRAPH

    import concourse.bacc as bacc
    import concourse.mybir as mybir

    dt = mybir.dt
    AF = mybir.ActivationFunctionType
    ALU = mybir.AluOpType
    AX = mybir.AxisListType

    nc = bacc.Bacc("TRN2")
    stk = ExitStack()

    sp_d = nc.declare_dram_parameter("sp", [22, NPAIR], dt.bfloat16, isOutput=False)
    idx_d = nc.declare_dram_parameter("idx", [NCHUNK * 128, 256], dt.int16, isOutput=False)
    tp_d = nc.declare_dram_parameter("tpad", [N_K * N_T, 64], dt.float32, isOutput=False)
    w_d = nc.declare_dram_parameter("wmat", [22, 128], dt.bfloat16, isOutput=False)
    lam_d = nc.declare_dram_parameter("lam", [2 * M_DIRS, N_C], dt.bfloat16, isOutput=False)
    out_d = nc.declare_dram_parameter("out", [1, 1], dt.float32, isOutput=True)

    sb = lambda name, shape, d: stk.enter_context(nc.sbuf_tensor(name, shape, d))
    ps = lambda name, shape: stk.enter_context(nc.psum_tensor(name, shape, dt.float32))
    sem = lambda name: stk.enter_context(nc.semaphore(name))

    PCH = CHUNK // 2  # 2048 pairs/chunk
    # double/triple buffered tiles
    spt = [sb(f"spt{i}", [22, PCH], dt.bfloat16) for i in range(3)]
    idxt = [sb(f"idxt{i}", [128, 256], dt.int16) for i in range(3)]
    tgt = [sb(f"tgt{i}", [128, 32, 64], dt.float32) for i in range(4)]
    phi = [sb(f"phi{i}", [128, PCH], dt.bfloat16) for i in range(3)]
    phb = [sb(f"phb{i}", [64, PCH], dt.bfloat16) for i in range(3)]
    Lt = [sb(f"Lt{i}", [128, 32, 32], dt.float32) for i in range(3)]
    Et = [sb(f"Et{i}", [128, 32, 32], dt.float32) for i in range(3)]
    dtm = [sb(f"dtm{i}", [128, 32], dt.float32) for i in range(3)]
    contrib = sb("contrib", [128, NCHUNK * 32], dt.float32)
    w_sb = sb("w_sb", [22, 128], dt.bfloat16)
    lam_sb = sb("lam_sb", [2 * M_DIRS, N_C], dt.bfloat16)
    ones_sb = sb("ones_sb", [128, 1], dt.float32)
    acc_sb = sb("acc_sb", [128, 1], dt.float32)
    res_sb = sb("res_sb", [1, 1], dt.float32)
    scr_a = sb("scr_a", [1, 1], dt.float32)
    scr_b = sb("scr_b", [1, 1], dt.float32)
    scr_p = ps("scr_p", [1, 1])
    import os as _os0
    STAGES = int(_os0.environ.get("STAGES", "99"))

    U = [ps(f"U{i}", [128, 512]) for i in range(4)]
    llv = [ps(f"llv{i}", [128, 512]) for i in range(2)]
    fin = ps("fin", [1, 1])

    s_lsp = [sem(f"s_lsp{i}") for i in range(3)]
    s_lix = [sem(f"s_lix{i}") for i in range(3)]
    s_gthv = [[sem(f"s_gth{i}_{g}") for g in range(4)] for i in range(4)]
    s_phb = [sem(f"s_phb{i}") for i in range(3)]
    s_out = sem("s_out"); s_mm1 = sem("s_mm1")
    s_sq = sem("s_sq"); s_mmf = sem("s_mmf"); s_add = sem("s_add")
    s_exp = sem("s_exp"); s_red = sem("s_red"); s_log = sem("s_log")
    s_acc = sem("s_acc"); s_fin = sem("s_fin"); s_res = sem("s_res")
    s_ini = sem("s_ini")

    import os as _os
    STUB_G = bool(_os.environ.get("STUB_GATHER"))
    C = NCHUNK

    with nc.Block() as block:

        @block.sync
        def _(e):
            e.dma_start(out=w_sb[:], in_=w_d[:]).then_inc(s_ini, 16)
            e.dma_start(out=lam_sb[:], in_=lam_d[:]).then_inc(s_ini, 16)
            for g in range(C + 2):
                if g < C:
                    c = g
                    if c >= 3:
                        e.wait_ge(s_mm1, 4 * (c - 2))              # sp buf reuse
                        for q in range(4):
                            e.wait_ge(s_gthv[(c - 3) % 4][q], 16 * ((c - 3) // 4 + 1))
                    e.dma_start(out=spt[c % 3][:], in_=sp_d[:, c * PCH:(c + 1) * PCH]
                                ).then_inc(s_lsp[c % 3], 16)
                    e.dma_start(out=idxt[c % 3][:], in_=idx_d[c * 128:(c + 1) * 128, :]
                                ).then_inc(s_lix[c % 3], 16)
                cp = g - 1
                if 0 <= cp < C:
                    e.wait_ge(s_sq, 4 * cp + 4)
                    if cp >= 3:
                        e.wait_ge(s_mmf, 32 * (cp - 2))            # phiB reuse
                    e.dma_start(out=phb[cp % 3][:], in_=phi[cp % 3][64:128, :]
                                ).then_inc(s_phb[cp % 3], 16)
            e.wait_ge(s_res, 1)
            e.dma_start(out=out_d[:], in_=res_sb[:]).then_inc(s_out, 16)
            e.wait_ge(s_out, 16)

        @block.gpsimd
        def _(e):
            for c in range(C):
                e.wait_ge(s_lix[c % 3], 16 * (c // 3 + 1))
                if c >= 4:
                    e.wait_ge(s_add, 2 * (c - 3))                  # tg buf reuse
                for g in range(4):
                    if STUB_G:
                        e.memset(tgt[c % 4][:, 8 * g:8 * (g + 1), :], 0.0
                                 ).then_inc(s_gthv[c % 4][g], 16)
                    else:
                        e.dma_gather(tgt[c % 4][:, 8 * g:8 * (g + 1), :], tp_d[:],
                                     idxt[c % 3][:, 64 * g:64 * (g + 1)],
                                     1024, 1024, 64).then_inc(s_gthv[c % 4][g], 16)

        @block.tensor
        def _(e):
            e.wait_ge(s_ini, 32)
            for g in range(C + 2):
                if g < C:
                    c = g
                    e.wait_ge(s_lsp[c % 3], 16 * (c // 3 + 1))
                    for j in range(4):
                        if c >= 1:
                            e.wait_ge(s_sq, 4 * (c - 1) + j + 1)   # U[j] reuse
                        e.matmul(U[j][:], w_sb[:],
                                 spt[c % 3][:, j * 512:(j + 1) * 512],
                                 start=True, stop=True).then_inc(s_mm1, 1)
                cm = g - 2
                if 0 <= cm < C:
                    for wdw in range(16):
                        b = wdw // 8
                        e.wait_ge(s_sq, 4 * cm + wdw // 4 + 1)
                        e.wait_ge(s_phb[cm % 3], 16 * (cm // 3 + 1))
                        if cm >= 1:
                            e.wait_ge(s_add, 2 * (cm - 1) + b + 1)  # llv reuse
                        for a in range(2):
                            k = 2 * (wdw % 8) + a
                            lhs = (phi[cm % 3][0:64, wdw * 128:(wdw + 1) * 128]
                                   if a == 0 else
                                   phb[cm % 3][0:64, wdw * 128:(wdw + 1) * 128])
                            e.matmul(llv[b][:, 32 * k:32 * (k + 1)], lhs,
                                     lam_sb[0:64, :],
                                     start=True, stop=True).then_inc(s_mmf, 1)
            e.wait_ge(s_acc, 1)
            e.matmul(fin[:], acc_sb[:], ones_sb[:],
                     start=True, stop=True).then_inc(s_fin, 1)

        @block.scalar
        def _(e):
            for t in range(C + 6):
                c = t - 1
                if 0 <= c < C:
                    for j in range(4):
                        e.wait_ge(s_mm1, 4 * c + j + 1)
                        if c >= 3:
                            e.wait_ge(s_mmf, 32 * (c - 2))          # phi buf reuse
                            e.wait_ge(s_phb[c % 3], 16 * ((c - 3) // 3 + 1))
                        e.activation(phi[c % 3][:, j * 512:(j + 1) * 512],
                                     U[j][:], AF.Square).then_inc(s_sq, 1)
                ce = t - 3
                if 0 <= ce < C:
                    for b in range(2):
                        e.wait_ge(s_add, 2 * ce + b + 1)
                        if ce >= 3:
                            e.wait_ge(s_red, 2 * (ce - 2))          # E buf reuse
                        e.activation(Et[ce % 3][:, 16 * b:16 * (b + 1), :],
                                     Lt[ce % 3][:, 16 * b:16 * (b + 1), :],
                                     AF.Exp).then_inc(s_exp, 1)
                cl = t - 5
                if 0 <= cl < C:
                    e.wait_ge(s_red, 2 * cl + 2)
                    e.activation(contrib[:, 32 * cl:32 * (cl + 1)],
                                 dtm[cl % 3][:], AF.Ln).then_inc(s_log, 1)
            e.wait_ge(s_fin, 1)
            e.activation(res_sb[:], fin[:], AF.Copy).then_inc(s_res, 1)

        @block.vector
        def _(e):
            e.memset(ones_sb[:], 1.0)
            for t in range(C + 5):
                ca = t - 3
                if 0 <= ca < C:
                    for b in range(2):
                        e.wait_ge(s_mmf, 32 * ca + 16 * (b + 1))
                        if b == 0:
                            for g in range(4):
                                e.wait_ge(s_gthv[ca % 4][g], 16 * (ca // 4 + 1))
                        if ca >= 3:
                            e.wait_ge(s_exp, 2 * (ca - 2))          # L buf reuse
                        e.tensor_tensor(Lt[ca % 3][:, 16 * b:16 * (b + 1), :],
                                        llv[b][:],
                                        tgt[ca % 4][:, 16 * b:16 * (b + 1), 0:32],
                                        ALU.add).then_inc(s_add, 1)
                cr = t - 4
                if 0 <= cr < C:
                    for b in range(2):
                        e.wait_ge(s_exp, 2 * cr + b + 1)
                        if cr >= 3:
                            e.wait_ge(s_log, cr - 2)                # dtm buf reuse
                        e.tensor_reduce(dtm[cr % 3][:, 16 * b:16 * (b + 1)],
                                        Et[cr % 3][:, 16 * b:16 * (b + 1), :],
                                        AX.X, ALU.add).then_inc(s_red, 1)
            e.wait_ge(s_log, C)
            e.tensor_reduce(acc_sb[:], contrib[:], AX.X, ALU.add).then_inc(s_acc, 1)

    nc.compile()
    _GRAPH = nc
    return nc


# ----------------------------------------------------------------------------
# entry point
# ----------------------------------------------------------------------------

LAST_RESULTS = None


def kernel(s, y, ks, ts, means, covs, b_mu, b_log_sig, beta_mu, beta_log_sig):
    import os
    global LAST_RESULTS
    s = np.asarray(s); y = np.asarray(y)
    ks = np.asarray(ks); ts = np.asarray(ts)
    means = np.asarray(means); covs = np.asarray(covs)
    b_mu = np.asarray(b_mu); b_log_sig = np.asarray(b_log_sig)
    beta_mu = np.asarray(beta_mu); beta_log_sig = np.asarray(beta_log_sig)

    prep = _host_prep(s, y, ks, ts, means, covs, b_mu,
                      b_log_sig, beta_mu, beta_log_sig)

    nc = _build_graph()
    from concourse.bass_utils import run_bass_kernel_spmd

    in_maps = []
    for i in range(N_CORES):
        in_maps.append({
            "sp": np.asarray(prep["sp_cores"][i]),
            "idx": prep["idx_cores"][i],
            "tpad": prep["Tpad"],
            "wmat": np.asarray(prep["W"]),
            "lam": np.asarray(prep["lam"]),
        })

    trace = bool(os.environ.get("BASS_TRACE"))
    res = run_bass_kernel_spmd(nc, in_maps, core_ids=list(range(N_CORES)),
                               trace=trace)
    LAST_RESULTS = res

    partials = [float(res.results[i]["out"].reshape(-1)[0]) for i in range(N_CORES)]
    total = (sum(partials)
             - N_CORES * NPAD_LOC * prep["lse_pad"]
             + prep["elbo_const"])
    return np.float32(total)



# revision 10
# speedup vs baseline: 6.9124x; 6.9124x over previous
"""Trainium2 Bass kernel for nn_ADVI (segment_reduce ELBO).

Math:
  elbo = const(prior - q) + sum_n LSE_c( ll[n,c] + log_pis[ks_n, c, ts_n] )
  log_pis[k,c,t] = b_c + beta[c,t]*y[k,t] - L[k,t]   (L = LSE_c of the first part)
  The -L[k,t] term is class-independent -> sum_n L[ks_n,ts_n] is computed on host.
  Remaining device math per spike:  A[n,c] = s~^T Pt_c s~ + g_n * beta[c, t_n]
  with s~ = [s;1], g_n = y[ks_n, ts_n], and Pt_c carrying b_c + all constants in
  its (10,10) entry.  The quadratic is fit EXACTLY (res ~2e-6) as
  sum_m lam[m,c] (v_m . s~)^2 over 62 shared directions; two extra exact
  "directions" ((g+1)/2)^2 and ((g-1)/2)^2 with coefficients +-beta[c,t]
  reconstruct g*beta.  Spikes are host-sorted into 128 t-buckets so each
  128-pair matmul window uses one lam_t; the window->t map is static and
  identical on all 8 cores (per-bucket window counts are globally padded).

  Device pipeline per chunk (4096 spikes = 2048 pair-columns, 2 spikes/col):
  DMA sp -> PE mm1 (proj to 128 dirs) -> square (ACT half / DVE half, ->bf16)
  -> PE mm2 vs lam_t table (out [128 pairs, 64] = A|B classes) -> ACT exp
  -> DVE segmented reduce (sum 32 classes) -> contrib.  One deferred Ln over
  all contribs at the end (avoids ACT table-set thrash), then reduce+matmul
  to a scalar.  No gather, no gpsimd work.
"""

import base64
import sys
from contextlib import ExitStack

import numpy as np

sys.path.insert(0, "/opt/trn_rl_repo")

import ml_dtypes  # noqa: E402

F16 = np.float16

LOG2PI = float(np.log(2.0 * np.pi))
N_K, N_T, N_C, N_D, N_SPK = 256, 128, 32, 10, 1000000
N_CORES = 8
CHUNK = 4096                      # spikes per chunk
WIN = 128                         # pairs per mm2 window (256 spikes)
M_DIRS = 62                       # fitted quadratic directions per spike

# 64 fitted directions (f64, 64x11) from the original reference fit; used as
# warm start for the 62-dir constrained refinement.
_V64_B64 = """
AAAAoH/q7z8AAADAZxKMPwAAAGC3gpK/AAAAYMKUkD8AAABAftuSvwAAAMD4rJq/AAAAIPyonj8AAABgqPmTPwAAAEAhMZI/AAAA
wOuImD8AAADgfAYIvwAAAAA0dIq/AAAAAG1L8D8AAABgiy+LvwAAAOChXpI/AAAAIJoyVj8AAACAp5SfPwAAACBKWni/AAAAQP9B
Qj8AAACA7E9svwAAAEAbVoO/AAAAAMFbk78AAABgyQqWPwAAAACRppK/AAAAgNFA7z8AAADgRMSUvwAAACAXYqE/AAAA4C9omL8A
AABgzk+MvwAAAACFkaE/AAAAgBYBkD8AAABAwQmbPwAAAMDHDY4/AAAAIAfUeL8AAACgFpiZvwAAAMB6ezK/AAAAoFOO8D8AAAAA
fSGRPwAAAMDzCpo/AAAAgFvYob8AAACAdd9zvwAAAAAHsYy/AAAAYDYjdj8AAAAgq6ybvwAAAGCAyZi/AAAAwORZeL8AAAAgkaqb
PwAAAGBoKJc/AAAAADpY7z8AAAAgFQabPwAAAGCR5Za/AAAAoN5dcT8AAADA7yyiPwAAACBJGWm/AAAAgMEakD8AAAAg5MGHPwAA
ACDI6om/AAAA4Bu6mr8AAAAghwSgvwAAAEBQuJ2/AAAAAIlT8D8AAACAoh2NvwAAAMDATKe/AAAAIPH4lj8AAACAN3JRPwAAAEA+
ipG/AAAA4MOIoj8AAADAWhqLvwAAAOAoZpA/AAAAIBbdkz8AAAAAv2GXvwAAAECqzJu/AAAAgDgG8D8AAAAAlYlaPwAAAGD6cIA/
AAAAgEdPmD8AAACAFWt9vwAAAGC7wZM/AAAAIG9KlL8AAABAXB2gPwAAACCKw5Q/AAAAwNc1hb8AAADAuhebPwAAAKAdFYg/AAAA
oImF8D8AAAAgVhyRvwAAAKAOMUU/AAAAoJFBq78AAADgFmd5PwAAAACdT5Q/AAAAwBoOlL8AAADAOPyOvwAAAOAhHpo/AAAAQJLF
k78AAADgQe+VPwAAAOBWfJq/AAAAgLgi8D8AAADA2yyCPwAAAOC96Y2/AAAAoCRGob8AAACASjBtPwAAACD7GIY/AAAAoLxHdj8A
AADA9/mDPwAAAGAT/YQ/AAAAQKxiZr8AAABAolWSvwAAAECaAnu/AAAAAJx/8D8AAABAzH2cvwAAACBK9+8/AAAAYAyajb8AAAAA
X8SUPwAAAGAMKZC/AAAAoF+YlD8AAADANBCYPwAAACCmSZK/AAAAwNllj78AAAAgHyeMvwAAAGBB1Zq/AAAAwHEo8D8AAACAiuyO
PwAAAGD4fO8/AAAAwJRTiz8AAADA1XWLvwAAAICtNHG/AAAA4Nbxn78AAADAWD1/PwAAAECEtD8/AAAAQPTfRT8AAABg852GPwAA
AIDISPA/AAAAwBRfkb8AAAAgOxJzPwAAAID6zO4/AAAAoJ/pfD8AAACA2zJ+vwAAAADpEoQ/AAAAAJQ7oj8AAAAAToeTvwAAAMDi
bYi/AAAAINCVj78AAACgrofwPwAAAMBaQ4w/AAAAoDJLnj8AAABAIxN8PwAAAOBBo/A/AAAAYCNFib8AAACgHnycvwAAAEDB5Kc/
AAAAIJG/jD8AAADgvKuBPwAAAABjioW/AAAAIId67j8AAABA91GXPwAAAEBYZU4/AAAAQHxIn78AAABgvBOWvwAAAIABS/A/AAAA
gNsAmL8AAABA1syXPwAAAOBNSW6/AAAAQEUUob8AAAAggTxivwAAACBQje8/AAAAoNnyhb8AAABg1N6DPwAAAGD3bmm/AAAAYCZT
mj8AAACg1kqdPwAAACBTo+4/AAAAAE4fij8AAADAnm+gPwAAAACXX4m/AAAA4HqUir8AAACAip7wPwAAAOCFKqK/AAAAIC0Qaz8A
AACgwpSavwAAAOC8aIy/AAAAACfalj8AAACg4viTPwAAAMBAte8/AAAAwHGml78AAABg6zaNvwAAAODNapC/AAAAgGk48D8AAADg
TGqcvwAAAID7tJE/AAAAYMYOob8AAAAg7A2RvwAAAIC4fWy/AAAAgNxWib8AAABgAteRvwAAACA2X+4/AAAAAKAnoD8AAAAAwJSG
vwAAAADtFPE/AAAAAA70hb8AAABAA9CTvwAAAKAxrpQ/AAAAoPUEkD8AAADAjSKYvwAAAIBnKJU/AAAAINyOmL8AAABA3Q6gPwAA
AKAsYPA/AAAAIOqql78AAACAPYzvPwAAAKAI3Zw/AAAAAOydYb8AAADAK5yAvwAAAEAQt32/AAAAoNLdhL8AAABg8g6UvwAAAOCo
cXI/AAAAIM91iz8AAADA9oyCPwAAAAAzT+8/AAAAgFRm8D8AAAAAf3PrPwAAAOBKGeI/AAAAgIl40r8AAADAE/TgvwAAAICSDrW/
AAAAIBOK6T8AAABgUrbMPwAAAIBGiuY/AAAAAIjZyD8AAACgLbjOPwAAAMB3EvI/AAAAYAxvor8AAACgqNzuPwAAAICA9ea/AAAA
4NskwT8AAAAAddHCPwAAACANYem/AAAAQMSt4L8AAAAgwJTHPwAAAIDdOqa/AAAAIIMe1j8AAADgOZmwvwAAAIAOZeE/AAAAgJKm
wj8AAACAZgDQvwAAAEDoc7m/AAAA4PbP1b8AAADA1VfYPwAAAICHW8o/AAAAoLXd3j8AAABAYhLLPwAAAKBdIrI/AAAAwHWv878A
AABgd4HjvwAAAICvlsm/AAAAAJkizr8AAADAI+LWvwAAAKBTx9e/AAAAADWflb8AAABgFzunPwAAAMARQa4/AAAAYIPZ5z8AAACg
mI/lPwAAAAAc66q/AAAAALOHxz8AAABgJ1boPwAAAKC4iNc/AAAAQByC4T8AAABADrHlvwAAAABrcry/AAAAILZ+5b8AAADAaXDw
vwAAAEDQHve/AAAAAEMNxr8AAABgc0f7PwAAAEAQHao/AAAA4Dnj2b8AAADgnjTJvwAAAIB5gNG/AAAAANUv4T8AAADgwC+jPwAA
AIC7adM/AAAAoCda4T8AAABA/CjwPwAAAOBztNA/AAAAAOFp+z8AAADgKvjJPwAAAABcaNw/AAAAgOFDtz8AAADgFR+xvwAAAMD2
ZPC/AAAAoE8h7z8AAABgIRzqPwAAAOAFl+E/AAAA4Gp7rr8AAABgbAHrPwAAAEDb37Y/AAAAIOWg578AAAAAiDruvwAAACD4pOS/
AAAAgKwc0L8AAABgtoHhvwAAAGAxoMS/AAAA4Nh81b8AAAAgfUPhvwAAAIBxJKy/AAAAoIOc7T8AAAAgiha4PwAAAGAu+/Y/AAAA
gIUH0T8AAABA88TWvwAAAMB9V+U/AAAA4PUWu78AAACAgZC+PwAAAEBuItO/AAAAYGIUnj8AAACA3EzyPwAAAGDVDtk/AAAAYP1t
+z8AAAAA7irtPwAAAOBVd7C/AAAAQGTjyL8AAAAgUcnpPwAAAIAnvqm/AAAAIJ8vUz8AAABAgQ65vwAAACB/GHw/AAAAoBK07T8A
AAAgVaDQPwAAAID8Pvu/AAAAQItmZr8AAAAAWuvzvwAAAAAM+MK/AAAA4CjN1j8AAAAAqDHVPwAAAIB858C/AAAAQHXvyD8AAACg
eYXxvwAAACBNCrS/AAAAAFOLxT8AAABgYzSyPwAAAMAM5uU/AAAAQFuAtD8AAAAgxdbUPwAAAEDha+W/AAAAQNZyzT8AAADA7iHq
PwAAAABZG+Y/AAAAAEi/1b8AAAAgVrDYvwAAAGC8oeM/AAAAgKM3fT8AAADgjq/VPwAAAEAIYu6/AAAAoFHN0j8AAABAFK+1PwAA
AKBrLcO/AAAAgHIKlz8AAADA83ruPwAAAEBkHMc/AAAAQP+iuz8AAACgIl3lvwAAAEBIWfW/AAAAoEg24D8AAABAD5DmvwAAAICR
8Lk/AAAAADnllr8AAACgAJrSvwAAAKBcgre/AAAAgMwU6D8AAAAgkqCqPwAAAEAjKr8/AAAAQF5h1L8AAAAgcib1PwAAAMAFHuM/
AAAAQF6b5D8AAABAI+3VPwAAAIBMt74/AAAA4AdRxD8AAABAVvnmPwAAAOCJStY/AAAAoM22xz8AAACgerzlvwAAAAB3Ooq/AAAA
oFWypT8AAACAkq3DPwAAAODLEdE/AAAAAMRs0j8AAADAFMXLvwAAAEDbuu2/AAAAoONnvj8AAABAJpSOvwAAAKBz8NI/AAAAAH7g
2j8AAACAmsXhPwAAAGCb8se/AAAAYIZf3j8AAAAAWEDqPwAAAOCGC6k/AAAAwBBN0j8AAADACEDlPwAAAGBDGd+/AAAAwAfY4L8A
AACgeYLxPwAAAICzEMG/AAAAICyI5r8AAADA3rv5PwAAAGDgAtK/AAAA4LpZ4r8AAABgj7ChvwAAAOCy0ti/AAAAQGyZ5r8AAABg
QX/RPwAAAKBhTcM/AAAAQG8S478AAABACtq8PwAAAOANg+I/AAAAAO96+T8AAACArrzlPwAAAIC+5PK/AAAAQG1Azr8AAACAkbCi
PwAAAEBzXuY/AAAA4Pkdrz8AAADgRKLMvwAAAAA7T8m/AAAAQM/C178AAADgrXfJvwAAAADkWbI/AAAAwMVS4r8AAAAA3GfWvwAA
AECtbde/AAAAYDdC2T8AAAAAYo7ivwAAAODOOui/AAAA4M31xj8AAACgxD+4vwAAACBvQtW/AAAAYIa/6r8AAABgqu2RvwAAAIBl
y/I/AAAAoL8Oyb8AAABgzknfvwAAAGB0D6s/AAAAYNkwkz8AAACghJPoPwAAAEAvFtI/AAAAoEz13D8AAABAPQLbPwAAACDeLbS/
AAAA4O/p+L8AAACA6QTyvwAAAED5JcU/AAAAoP6L0z8AAABgdP9XPwAAAKARV8e/AAAAgDaj6L8AAADAeTjdvwAAAABgQNS/AAAA
AGOR4b8AAADABMjDvwAAAAAowPi/AAAAoEGt0L8AAAAgRU3LvwAAAGCA7+C/AAAAAB5H2L8AAAAg6J/evwAAAOAxENg/AAAAQIOU
vL8AAAAgmSnivwAAAEBeHrg/AAAAgOpb0r8AAADASHCsvwAAAACKJs2/AAAAgGp54j8AAAAA3b3DvwAAAMBZXEe/AAAAQFR11r8A
AACAzuKgPwAAAOCcTOc/AAAAwJVtsz8AAABg7iiqvwAAAMCVEdE/AAAAIOtXlb8AAABgNh/avwAAAEAXUOC/AAAA4FG15L8AAAAg
60eivwAAAMCEVto/AAAAYFLJlr8AAACgHCPwPwAAAKBxsOg/AAAAYKygzj8AAADAH4zGvwAAACBmbvm/AAAAADI4zz8AAABg9RXh
PwAAAKAeiNc/AAAAYP6fwT8AAABgbOfTvwAAACAkSsU/AAAAYD527r8AAABgxBLvvwAAACBxttG/AAAAgNkvxT8AAACgxQb6vwAA
AICUDNs/AAAAwIM9xz8AAADgLjXiPwAAAEDbas+/AAAAYCUQ1L8AAACghOfAvwAAAOBB07c/AAAAwNAs2D8AAADgjBqjvwAAAGDZ
kum/AAAAYP0rwb8AAACg2zXHPwAAAOD3MX2/AAAAwHcypD8AAABgbZjePwAAAEBJb78/AAAA4Ky6zj8AAABACfHaPwAAAGAOpeC/
AAAAgAnHyT8AAABA66vWvwAAAECJtLI/AAAAAJWywL8AAABA9mXzvwAAAKD9H+6/AAAAgJ0S0j8AAABA2ZvZvwAAAABlAJQ/AAAA
wKY64T8AAAAgRSvQPwAAAACo6ts/AAAAoIBK7L8AAACgUCD+vwAAAKATcrU/AAAAwB0u8L8AAAAAJCXqvwAAAIBCbt0/AAAAIKXe
4L8AAABANdC1vwAAAGBGy9g/AAAAoNoDxT8AAAAAeK7fPwAAAGAYIua/AAAAgJsM/j8AAABAOVXYvwAAAEC/6cK/AAAAQMeT5L8A
AADAzirBvwAAAGA0m6k/AAAAwDXc478AAACAd9LevwAAACB6O9g/AAAAQOey8b8AAABAsSu6PwAAAGAIPZy/AAAAYA+fuz8AAACg
wnCzvwAAACCzI+E/AAAAwG4KnL8AAACg+vnyvwAAAIBVn8s/AAAAQJP0pD8AAACAH6LhPwAAAOCREtG/AAAAgOL9xT8AAADAYi22
vwAAAADJgPI/AAAAwHsd6z8AAACAYtjhPwAAACA/P9s/AAAAoLEF+D8AAADgNBjhvwAAACCPFdq/AAAAoDMWcT8AAADgaCHjPwAA
ACAtf8G/AAAAQDKGAEAAAADge4XovwAAAEBblN+/AAAAgHQe0L8AAAAgEmDMvwAAAEAhM/O/AAAAwCKf3z8AAADAsK/cPwAAAADc
xYm/AAAAwGea4L8AAAAgsj/DPwAAAIDCbgBAAAAAAATR9L8AAACgvTnYPwAAAICb38g/AAAAYEgNyb8AAADgLpm3PwAAAODNSt2/
AAAA4Dnqy78AAABgb2rkPwAAAADV9/I/AAAAoFGq1T8AAACgs7asPwAAAIA6UKW/AAAAAIsUxD8AAADAQdLmPwAAAMDJnNG/AAAA
ICnNt78AAADABcflPwAAAGCr754/AAAAwJXC6z8AAADgH1G2vwAAAGARP/a/AAAAgA9Pvb8AAACgcTrkvwAAAEB3Y8W/AAAAgDXO
8z8AAABgajPkvwAAAKBIM+m/AAAAANPO6D8AAACAOx7APwAAAAA52r+/AAAAQGFD4b8AAADAXdzwPwAAAAAGTwDAAAAAIKQU4b8A
AABg6OTIvwAAAODaBfA/AAAAgCVb2r8AAAAAw/3hvwAAAKCqAeM/AAAAwOnRij8AAADAoCLMvwAAAKBWaea/AAAA4P+q7T8AAACA
OBUAQAAAAECR/ty/AAAAoGzW5b8AAABgYS6wvwAAAOBvJKa/AAAAwN6H0D8AAAAA56zhvwAAAMDFM9a/AAAAwBsT7r8AAAAARH/Q
vwAAAOBbjLe/AAAAwHpZkL8AAACATezjvwAAAECzEHQ/AAAAQMvO5b8AAABA7VJgvwAAAGCtpuG/AAAAIG5Byb8AAAAgLjisvwAA
AOAhb8E/AAAAIN/o4j8AAACgR33iPwAAAMDPE4i/AAAAALb/zD8AAAAgUa3UPwAAAMAJR8C/AAAAAIAZ4r8AAAAAF4u7PwAAACAa
ruI/AAAAABfXzr8AAACAzfzyvwAAAKCUeN2/AAAAgL/57L8AAABA7kv7PwAAAEB7uNi/AAAAINzb0r8AAADg637OPwAAAIAzQNs/
AAAAgD/Hp78AAABgWzXYvwAAAMDUusQ/AAAAwMHP8T8AAADA4SjcPwAAAKCMA+k/AAAAwESr/D8AAACgSeiqvwAAAMBhpt0/AAAA
IGnxzz8AAACAXmPcPwAAACD+XKC/AAAAIOwc6D8AAAAAc+rePwAAAEALMqG/AAAA4F05ub8AAADAAhPIPwAAAAAp94i/AAAAYFG5
1r8AAAAAtHSTvwAAAEBUo+G/AAAA4ErdhD8AAADg7YrkPwAAAKARbMw/AAAAoCG9vr8AAADgnoyuPwAAAEAXH7M/AAAAIM5VoD8A
AACgMD2HPw==
"""


def _decode_v():
    b = base64.b64decode("".join(_V64_B64.split()))
    return np.frombuffer(b, dtype=np.float64).reshape(64, 11).copy()


# ----------------------------------------------------------------------------
# host-side math prep (small params + O(n) packing; no O(n) flops beyond
# gathers/sorts)
# ----------------------------------------------------------------------------

def _fit_dirs(P_t):
    """Fit 62 directions V (62,11) and Lam (62,32) such that
    sum_m Lam[m,c] (v_m . x)^2 ~= x^T P_t[c] x with ALL entries constrained
    (including (10,10)).  Warm-started from the stored 64-dir fit."""
    V0 = _decode_v()
    Pf = P_t.reshape(N_C, -1)                     # (32,121)
    scale = np.linalg.norm(Pf)

    def lam_and_res(V):
        B = np.einsum('mi,mj->mij', V, V).reshape(len(V), -1)
        Lam, *_ = np.linalg.lstsq(B.T, Pf.T, rcond=None)
        R = B.T @ Lam - Pf.T                      # (121,32)
        return Lam, R, B

    Lam64, _, _ = lam_and_res(V0)
    keep = np.argsort(-np.abs(Lam64).sum(axis=1))[:M_DIRS]
    V = V0[keep].copy()

    Lam, R, B = lam_and_res(V)
    m = np.zeros_like(V); v2 = np.zeros_like(V)
    lr, b1, b2, eps = 3e-3, 0.9, 0.999, 1e-8
    best = (np.inf, V.copy(), Lam)
    for it in range(8000):
        Lam, R, B = lam_and_res(V)
        loss = float(np.sum(R * R))
        if loss < best[0]:
            best = (loss, V.copy(), Lam)
        if np.sqrt(loss) / scale < 2e-6:
            break
        Rc = (R.T).reshape(N_C, 11, 11)
        G = np.einsum('mc,cij,mj->mi', Lam, Rc + Rc.transpose(0, 2, 1), V)
        m = b1 * m + (1 - b1) * G
        v2 = b2 * v2 + (1 - b2) * G * G
        mh = m / (1 - b1 ** (it + 1)); vh = v2 / (1 - b2 ** (it + 1))
        V = V - lr * mh / (np.sqrt(vh) + eps)
    loss, V, Lam = best
    return V, Lam, np.sqrt(loss) / scale


def _host_prep(s, y, ks, ts, means, covs, b_mu, b_log_sig, beta_mu, beta_log_sig):
    f8 = np.float64
    means8, covs8 = means.astype(f8), covs.astype(f8)
    P = np.linalg.inv(covs8)
    P = 0.5 * (P + P.transpose(0, 2, 1))
    sign, logdet = np.linalg.slogdet(covs8)
    assert np.all(sign > 0)

    # P_tilde (32,11,11): s~^T Pt s~ = ll[c](s) + b_c  (constants in (10,10))
    w = np.einsum('cij,cj->ci', P, means8)
    muPmu = np.einsum('ci,cij,cj->c', means8, P, means8)
    Kc = -0.5 * muPmu - 0.5 * logdet - 0.5 * N_D * LOG2PI
    Pt = np.zeros((N_C, 11, 11))
    Pt[:, :10, :10] = -0.5 * P
    Pt[:, :10, 10] = 0.5 * w
    Pt[:, 10, :10] = 0.5 * w
    Pt1010 = Kc + b_mu.astype(f8)

    # overflow shift: device logits A <= max_c(Pt1010_c + g*beta_ct); keep
    # exp(A) well inside f16 range by subtracting S from every class const.
    ks64 = ks.astype(np.int64); ts64 = ts.astype(np.int64)
    g_all = y[ks64, ts64].astype(f8)                          # (n,)
    beta8 = beta_mu.astype(f8)                                # (32,128)
    bound = (Pt1010[None, :] + g_all[:, None] * beta8[:, ts64].T).max(axis=1)
    S = float(max(0.0, bound.max() - 80.0))
    Pt[:, 10, 10] = Pt1010 - S

    V, Lam, res = _fit_dirs(Pt)

    # per-(k,t) LSE table L (f64) and its sum over spikes (host-only term)
    y8 = y.astype(f8)
    ll_kct = b_mu.astype(f8)[None, :, None] + \
        beta_mu.astype(f8)[None, :, :] * y8[:, None, :]      # (256,32,128)
    mx = ll_kct.max(axis=1)
    L = mx + np.log(np.exp(ll_kct - mx[:, None, :]).sum(axis=1))  # (256,128)
    L_sum = float(L[ks64, ts64].sum())

    # prior - q const (f64, formulas of the reference)
    lp = -0.5 * (b_mu.astype(f8) ** 2 + LOG2PI).sum() \
         - 0.5 * (beta_mu.astype(f8) ** 2 + LOG2PI).sum()
    lq = (-0.5 * LOG2PI * b_mu.size - b_log_sig.astype(f8).sum()) + \
         (-0.5 * LOG2PI * beta_mu.size - beta_log_sig.astype(f8).sum())
    elbo_const = lp - lq

    # --- bucket spikes by t; static window->t map shared by all cores ---
    order = np.argsort(ts64, kind='stable')
    counts = np.bincount(ts64, minlength=N_T)                 # (128,)
    w_t = np.maximum(1, -(-counts // (N_CORES * 2 * WIN)))    # windows/core/t
    T_wins = np.repeat(np.arange(N_T), w_t)                   # per-core windows
    n_win = len(T_wins)
    n_win_pad = -(-n_win // 16) * 16
    C = n_win_pad // 16
    T_wins = np.concatenate([T_wins, np.zeros(n_win_pad - n_win, np.int64)])
    NLOC = C * CHUNK                                          # spikes per core

    # window start offsets per bucket (in spikes, per core)
    starts = np.concatenate([[0], np.cumsum(w_t) * 2 * WIN])  # (129,)

    s_aug = np.zeros((N_SPK, 12), dtype=np.float32)
    s_aug[:, :10] = s
    s_aug[:, 10] = 1.0
    s_aug[:, 11] = g_all

    sp_cores = []
    n_real = np.zeros(N_CORES, np.int64)
    pos = 0
    rec = [np.zeros((NLOC, 12), dtype=np.float32) for _ in range(N_CORES)]
    for t in range(N_T):
        idx_t = order[pos:pos + counts[t]]
        pos += counts[t]
        splits = np.array_split(idx_t, N_CORES)
        for i in range(N_CORES):
            k = len(splits[i])
            rec[i][starts[t]:starts[t] + k] = s_aug[splits[i]]
            n_real[i] += k
    for i in range(N_CORES):
        # pack pairs: (npair, 2, 12) -> (24, npair)
        spk = rec[i].reshape(NLOC // 2, 2, 12).transpose(1, 2, 0).reshape(24, NLOC // 2)
        sp_cores.append(spk.astype(F16))
    pads_total = int(N_CORES * NLOC - n_real.sum())

    # W stationary (24, 128) bf16: per spike 62 quad dirs + 2 g-dirs
    W = np.zeros((24, 128), dtype=np.float32)
    for h in range(2):                                        # A/B half
        r0, c0 = 12 * h, 64 * h
        W[r0:r0 + 11, c0:c0 + M_DIRS] = V.T.astype(np.float32)
        W[r0 + 10, c0 + 62] = 0.5; W[r0 + 11, c0 + 62] = 0.5   # (1+g)/2
        W[r0 + 10, c0 + 63] = -0.5; W[r0 + 11, c0 + 63] = 0.5  # (g-1)/2

    # lam table (128, 128*64) bf16: per t a (128,64) block, block-diag A|B
    lam_all = np.zeros((128, N_T * 64), dtype=np.float32)
    Lam32 = Lam.astype(np.float32)                            # (62,32)
    beta32 = beta_mu.astype(np.float32)                       # (32,128)
    for t in range(N_T):
        blk = lam_all[:, 64 * t:64 * (t + 1)]
        blk[0:M_DIRS, 0:32] = Lam32
        blk[62, 0:32] = beta32[:, t]
        blk[63, 0:32] = -beta32[:, t]
        blk[64:64 + M_DIRS, 32:64] = Lam32
        blk[126, 32:64] = beta32[:, t]
        blk[127, 32:64] = -beta32[:, t]

    return dict(W=W.astype(F16), lam_all=lam_all.astype(F16),
                sp_cores=sp_cores, T_wins=tuple(int(t) for t in T_wins),
                C=C, pads_total=pads_total, L_sum=L_sum, S=S,
                n_real=int(n_real.sum()), elbo_const=elbo_const, fit_res=res)


# ----------------------------------------------------------------------------
# device graph
# ----------------------------------------------------------------------------

_GRAPHS = {}


def _build_graph(C, T_wins):
    key = (C, T_wins)
    if key in _GRAPHS:
        return _GRAPHS[key]

    import concourse.bacc as bacc
    import concourse.mybir as mybir

    dt = mybir.dt
    AF = mybir.ActivationFunctionType
    ALU = mybir.AluOpType
    AX = mybir.AxisListType

    nc = bacc.Bacc("TRN2")
    stk = ExitStack()

    NPAIR = C * CHUNK // 2
    sp_d = nc.declare_dram_parameter("sp", [24, NPAIR], dt.float16, isOutput=False)
    w_d = nc.declare_dram_parameter("wmat", [24, 128], dt.float16, isOutput=False)
    lam_d = nc.declare_dram_parameter("lam", [128, N_T * 64], dt.float16,
                                      isOutput=False)
    out_d = nc.declare_dram_parameter("out", [128, 1], dt.float32, isOutput=True)

    sb = lambda name, shape, d: stk.enter_context(nc.sbuf_tensor(name, shape, d))
    ps = lambda name, shape: stk.enter_context(nc.psum_tensor(name, shape, dt.float32))
    sem = lambda name: stk.enter_context(nc.semaphore(name))

    PCH = CHUNK // 2                  # 2048 pairs per chunk
    spt = [sb(f"spt{i}", [24, PCH], dt.float16) for i in range(3)]
    phi = [sb(f"phi{i}", [128, PCH], dt.float16) for i in range(3)]
    E = [sb(f"E{i}", [128, 32, 32], dt.float32) for i in range(2)]
    contrib = sb("contrib", [128, C * 32], dt.float32)
    lncon = sb("lncon", [128, C * 32], dt.float32)
    w_sb = sb("w_sb", [24, 128], dt.float16)
    lam_sb = sb("lam_sb", [128, N_T * 64], dt.float16)
    acc_sb = sb("acc_sb", [128, 1], dt.float32)

    U = ps("U", [128, 2048])
    llv = [ps(f"llv{i}", [128, 1024]) for i in range(2)]

    s_lsp = [sem(f"s_lsp{i}") for i in range(3)]
    s_inw = sem("s_inw"); s_inl = sem("s_inl")
    s_mm1 = sem("s_mm1"); s_mmf = sem("s_mmf")
    s_sq = sem("s_sq"); s_exp = sem("s_exp"); s_red = sem("s_red")
    s_log = sem("s_log"); s_acc = sem("s_acc"); s_out = sem("s_out")

    with nc.Block() as block:

        @block.sync
        def _(e):
            e.dma_start(out=w_sb[:], in_=w_d[:]).then_inc(s_inw, 16)
            e.dma_start(out=lam_sb[:], in_=lam_d[:]).then_inc(s_inl, 16)
            for c in range(C):
                if c >= 3:
                    e.wait_ge(s_mm1, 4 * (c - 3) + 4)         # spt buf reuse
                e.dma_start(out=spt[c % 3][:], in_=sp_d[:, c * PCH:(c + 1) * PCH]
                            ).then_inc(s_lsp[c % 3], 16)
            e.wait_ge(s_acc, 1)
            e.dma_start(out=out_d[:], in_=acc_sb[:]).then_inc(s_out, 16)
            e.wait_ge(s_out, 16)

        @block.tensor
        def _(e):
            e.wait_ge(s_inw, 16)
            for g in range(C + 2):
                cm = g - 2
                if cm == 0:
                    e.wait_ge(s_inl, 16)                       # lam table loaded
                if 0 <= cm < C:
                    e.wait_ge(s_sq, cm + 1)                    # phi(cm) ready
                    if cm >= 2:
                        e.wait_ge(s_exp, cm - 1)               # llv buf reuse
                    for wd in range(16):
                        t = T_wins[16 * cm + wd]
                        e.matmul(llv[cm % 2][:, 64 * wd:64 * wd + 64],
                                 phi[cm % 3][:, 128 * wd:128 * (wd + 1)],
                                 lam_sb[:, 64 * t:64 * t + 64],
                                 start=True, stop=True).then_inc(s_mmf, 1)
                if g < C:
                    c = g
                    e.wait_ge(s_lsp[c % 3], 16 * (c // 3 + 1))
                    if c >= 1:
                        e.wait_ge(s_sq, c)                     # U reuse
                    for j in range(4):
                        e.matmul(U[:, j * 512:(j + 1) * 512],
                                 w_sb[:], spt[c % 3][:, j * 512:(j + 1) * 512],
                                 start=True, stop=True).then_inc(s_mm1, 1)

        @block.scalar
        def _(e):
            for g in range(C + 3):
                c1 = g - 1
                if 0 <= c1 < C:                                # square
                    e.wait_ge(s_mm1, 4 * c1 + 4)
                    if c1 >= 3:                                # phi buf reuse
                        e.wait_ge(s_mmf, 16 * (c1 - 2))
                    e.activation(phi[c1 % 3][:], U[:],
                                 AF.Square).then_inc(s_sq, 1)
                c2 = g - 3
                if 0 <= c2 < C:                                # exp
                    e.wait_ge(s_mmf, 16 * c2 + 16)
                    if c2 >= 2:                                # E buf reuse
                        e.wait_ge(s_red, c2 - 1)
                    e.activation(E[c2 % 2][:], llv[c2 % 2][:],
                                 AF.Exp).then_inc(s_exp, 1)
            e.wait_ge(s_red, C)
            e.activation(lncon[:], contrib[:], AF.Ln).then_inc(s_log, 1)

        @block.vector
        def _(e):
            for g in range(C + 4):
                cr = g - 4
                if 0 <= cr < C:                                # segmented reduce
                    e.wait_ge(s_exp, cr + 1)
                    e.tensor_reduce(contrib[:, 32 * cr:32 * cr + 32],
                                    E[cr % 2][:], AX.X, ALU.add
                                    ).then_inc(s_red, 1)
            e.wait_ge(s_log, 1)
            e.tensor_reduce(acc_sb[:], lncon[:], AX.X, ALU.add).then_inc(s_acc, 1)

    nc.compile()
    _GRAPHS[key] = nc
    return nc


# ----------------------------------------------------------------------------
# entry point
# ----------------------------------------------------------------------------

LAST_RESULTS = None


def kernel(s, y, ks, ts, means, covs, b_mu, b_log_sig, beta_mu, beta_log_sig):
    import os
    global LAST_RESULTS
    s = np.asarray(s); y = np.asarray(y)
    ks = np.asarray(ks); ts = np.asarray(ts)
    means = np.asarray(means); covs = np.asarray(covs)
    b_mu = np.asarray(b_mu); b_log_sig = np.asarray(b_log_sig)
    beta_mu = np.asarray(beta_mu); beta_log_sig = np.asarray(beta_log_sig)

    prep = _host_prep(s, y, ks, ts, means, covs, b_mu,
                      b_log_sig, beta_mu, beta_log_sig)

    nc = _build_graph(prep["C"], prep["T_wins"])
    from concourse.bass_utils import run_bass_kernel_spmd

    in_maps = []
    for i in range(N_CORES):
        in_maps.append({
            "sp": np.asarray(prep["sp_cores"][i]),
            "wmat": np.asarray(prep["W"]),
            "lam": np.asarray(prep["lam_all"]),
        })

    trace = bool(os.environ.get("BASS_TRACE"))
    res = run_bass_kernel_spmd(nc, in_maps, core_ids=list(range(N_CORES)),
                               trace=trace)
    LAST_RESULTS = res

    partials = [float(res.results[i]["out"].astype(np.float64).sum())
                for i in range(N_CORES)]
    total = (sum(partials)
             + prep["S"] * prep["n_real"]
             - prep["pads_total"] * float(np.log(32.0))
             - prep["L_sum"]
             + prep["elbo_const"])
    return np.float32(total)


# revision 12
# speedup vs baseline: 7.5757x; 1.0960x over previous
"""Trainium2 Bass kernel for nn_ADVI (segment_reduce ELBO).

Math:
  elbo = const(prior - q) + sum_n LSE_c( ll[n,c] + log_pis[ks_n, c, ts_n] )
  log_pis[k,c,t] = b_c + beta[c,t]*y[k,t] - L[k,t]   (L = LSE_c of the first part)
  The -L[k,t] term is class-independent -> sum_n L[ks_n,ts_n] is computed on host.
  Remaining device math per spike:  A[n,c] = s~^T Pt_c s~ + g_n * beta[c, t_n]
  with s~ = [s;1], g_n = y[ks_n, ts_n], and Pt_c carrying b_c + all constants in
  its (10,10) entry.  The quadratic is fit EXACTLY (res ~2e-6) as
  sum_m lam[m,c] (v_m . s~)^2 over 62 shared directions; two extra exact
  "directions" ((g+1)/2)^2 and ((g-1)/2)^2 with coefficients +-beta[c,t]
  reconstruct g*beta.  Spikes are host-sorted into 128 t-buckets so each
  128-pair matmul window uses one lam_t; the window->t map is static and
  identical on all 8 cores (per-bucket window counts are globally padded).

  Device pipeline per chunk (4096 spikes = 2048 pair-columns, 2 spikes/col):
  DMA sp -> PE mm1 (proj to 128 dirs) -> square (ACT half / DVE half, ->bf16)
  -> PE mm2 vs lam_t table (out [128 pairs, 64] = A|B classes) -> ACT exp
  -> DVE segmented reduce (sum 32 classes) -> contrib.  One deferred Ln over
  all contribs at the end (avoids ACT table-set thrash), then reduce+matmul
  to a scalar.  No gather, no gpsimd work.
"""

import base64
import sys
from contextlib import ExitStack

import numpy as np

sys.path.insert(0, "/opt/trn_rl_repo")

import ml_dtypes  # noqa: E402

F16 = np.float16

LOG2PI = float(np.log(2.0 * np.pi))
N_K, N_T, N_C, N_D, N_SPK = 256, 128, 32, 10, 1000000
N_CORES = 8
CHUNK = 4096                      # spikes per chunk
WIN = 128                         # pairs per mm2 window (256 spikes)
M_DIRS = 62                       # fitted quadratic directions per spike

# 64 fitted directions (f64, 64x11) from the original reference fit; used as
# warm start for the 62-dir constrained refinement.
_V64_B64 = """
AAAAoH/q7z8AAADAZxKMPwAAAGC3gpK/AAAAYMKUkD8AAABAftuSvwAAAMD4rJq/AAAAIPyonj8AAABgqPmTPwAAAEAhMZI/AAAA
wOuImD8AAADgfAYIvwAAAAA0dIq/AAAAAG1L8D8AAABgiy+LvwAAAOChXpI/AAAAIJoyVj8AAACAp5SfPwAAACBKWni/AAAAQP9B
Qj8AAACA7E9svwAAAEAbVoO/AAAAAMFbk78AAABgyQqWPwAAAACRppK/AAAAgNFA7z8AAADgRMSUvwAAACAXYqE/AAAA4C9omL8A
AABgzk+MvwAAAACFkaE/AAAAgBYBkD8AAABAwQmbPwAAAMDHDY4/AAAAIAfUeL8AAACgFpiZvwAAAMB6ezK/AAAAoFOO8D8AAAAA
fSGRPwAAAMDzCpo/AAAAgFvYob8AAACAdd9zvwAAAAAHsYy/AAAAYDYjdj8AAAAgq6ybvwAAAGCAyZi/AAAAwORZeL8AAAAgkaqb
PwAAAGBoKJc/AAAAADpY7z8AAAAgFQabPwAAAGCR5Za/AAAAoN5dcT8AAADA7yyiPwAAACBJGWm/AAAAgMEakD8AAAAg5MGHPwAA
ACDI6om/AAAA4Bu6mr8AAAAghwSgvwAAAEBQuJ2/AAAAAIlT8D8AAACAoh2NvwAAAMDATKe/AAAAIPH4lj8AAACAN3JRPwAAAEA+
ipG/AAAA4MOIoj8AAADAWhqLvwAAAOAoZpA/AAAAIBbdkz8AAAAAv2GXvwAAAECqzJu/AAAAgDgG8D8AAAAAlYlaPwAAAGD6cIA/
AAAAgEdPmD8AAACAFWt9vwAAAGC7wZM/AAAAIG9KlL8AAABAXB2gPwAAACCKw5Q/AAAAwNc1hb8AAADAuhebPwAAAKAdFYg/AAAA
oImF8D8AAAAgVhyRvwAAAKAOMUU/AAAAoJFBq78AAADgFmd5PwAAAACdT5Q/AAAAwBoOlL8AAADAOPyOvwAAAOAhHpo/AAAAQJLF
k78AAADgQe+VPwAAAOBWfJq/AAAAgLgi8D8AAADA2yyCPwAAAOC96Y2/AAAAoCRGob8AAACASjBtPwAAACD7GIY/AAAAoLxHdj8A
AADA9/mDPwAAAGAT/YQ/AAAAQKxiZr8AAABAolWSvwAAAECaAnu/AAAAAJx/8D8AAABAzH2cvwAAACBK9+8/AAAAYAyajb8AAAAA
X8SUPwAAAGAMKZC/AAAAoF+YlD8AAADANBCYPwAAACCmSZK/AAAAwNllj78AAAAgHyeMvwAAAGBB1Zq/AAAAwHEo8D8AAACAiuyO
PwAAAGD4fO8/AAAAwJRTiz8AAADA1XWLvwAAAICtNHG/AAAA4Nbxn78AAADAWD1/PwAAAECEtD8/AAAAQPTfRT8AAABg852GPwAA
AIDISPA/AAAAwBRfkb8AAAAgOxJzPwAAAID6zO4/AAAAoJ/pfD8AAACA2zJ+vwAAAADpEoQ/AAAAAJQ7oj8AAAAAToeTvwAAAMDi
bYi/AAAAINCVj78AAACgrofwPwAAAMBaQ4w/AAAAoDJLnj8AAABAIxN8PwAAAOBBo/A/AAAAYCNFib8AAACgHnycvwAAAEDB5Kc/
AAAAIJG/jD8AAADgvKuBPwAAAABjioW/AAAAIId67j8AAABA91GXPwAAAEBYZU4/AAAAQHxIn78AAABgvBOWvwAAAIABS/A/AAAA
gNsAmL8AAABA1syXPwAAAOBNSW6/AAAAQEUUob8AAAAggTxivwAAACBQje8/AAAAoNnyhb8AAABg1N6DPwAAAGD3bmm/AAAAYCZT
mj8AAACg1kqdPwAAACBTo+4/AAAAAE4fij8AAADAnm+gPwAAAACXX4m/AAAA4HqUir8AAACAip7wPwAAAOCFKqK/AAAAIC0Qaz8A
AACgwpSavwAAAOC8aIy/AAAAACfalj8AAACg4viTPwAAAMBAte8/AAAAwHGml78AAABg6zaNvwAAAODNapC/AAAAgGk48D8AAADg
TGqcvwAAAID7tJE/AAAAYMYOob8AAAAg7A2RvwAAAIC4fWy/AAAAgNxWib8AAABgAteRvwAAACA2X+4/AAAAAKAnoD8AAAAAwJSG
vwAAAADtFPE/AAAAAA70hb8AAABAA9CTvwAAAKAxrpQ/AAAAoPUEkD8AAADAjSKYvwAAAIBnKJU/AAAAINyOmL8AAABA3Q6gPwAA
AKAsYPA/AAAAIOqql78AAACAPYzvPwAAAKAI3Zw/AAAAAOydYb8AAADAK5yAvwAAAEAQt32/AAAAoNLdhL8AAABg8g6UvwAAAOCo
cXI/AAAAIM91iz8AAADA9oyCPwAAAAAzT+8/AAAAgFRm8D8AAAAAf3PrPwAAAOBKGeI/AAAAgIl40r8AAADAE/TgvwAAAICSDrW/
AAAAIBOK6T8AAABgUrbMPwAAAIBGiuY/AAAAAIjZyD8AAACgLbjOPwAAAMB3EvI/AAAAYAxvor8AAACgqNzuPwAAAICA9ea/AAAA
4NskwT8AAAAAddHCPwAAACANYem/AAAAQMSt4L8AAAAgwJTHPwAAAIDdOqa/AAAAIIMe1j8AAADgOZmwvwAAAIAOZeE/AAAAgJKm
wj8AAACAZgDQvwAAAEDoc7m/AAAA4PbP1b8AAADA1VfYPwAAAICHW8o/AAAAoLXd3j8AAABAYhLLPwAAAKBdIrI/AAAAwHWv878A
AABgd4HjvwAAAICvlsm/AAAAAJkizr8AAADAI+LWvwAAAKBTx9e/AAAAADWflb8AAABgFzunPwAAAMARQa4/AAAAYIPZ5z8AAACg
mI/lPwAAAAAc66q/AAAAALOHxz8AAABgJ1boPwAAAKC4iNc/AAAAQByC4T8AAABADrHlvwAAAABrcry/AAAAILZ+5b8AAADAaXDw
vwAAAEDQHve/AAAAAEMNxr8AAABgc0f7PwAAAEAQHao/AAAA4Dnj2b8AAADgnjTJvwAAAIB5gNG/AAAAANUv4T8AAADgwC+jPwAA
AIC7adM/AAAAoCda4T8AAABA/CjwPwAAAOBztNA/AAAAAOFp+z8AAADgKvjJPwAAAABcaNw/AAAAgOFDtz8AAADgFR+xvwAAAMD2
ZPC/AAAAoE8h7z8AAABgIRzqPwAAAOAFl+E/AAAA4Gp7rr8AAABgbAHrPwAAAEDb37Y/AAAAIOWg578AAAAAiDruvwAAACD4pOS/
AAAAgKwc0L8AAABgtoHhvwAAAGAxoMS/AAAA4Nh81b8AAAAgfUPhvwAAAIBxJKy/AAAAoIOc7T8AAAAgiha4PwAAAGAu+/Y/AAAA
gIUH0T8AAABA88TWvwAAAMB9V+U/AAAA4PUWu78AAACAgZC+PwAAAEBuItO/AAAAYGIUnj8AAACA3EzyPwAAAGDVDtk/AAAAYP1t
+z8AAAAA7irtPwAAAOBVd7C/AAAAQGTjyL8AAAAgUcnpPwAAAIAnvqm/AAAAIJ8vUz8AAABAgQ65vwAAACB/GHw/AAAAoBK07T8A
AAAgVaDQPwAAAID8Pvu/AAAAQItmZr8AAAAAWuvzvwAAAAAM+MK/AAAA4CjN1j8AAAAAqDHVPwAAAIB858C/AAAAQHXvyD8AAACg
eYXxvwAAACBNCrS/AAAAAFOLxT8AAABgYzSyPwAAAMAM5uU/AAAAQFuAtD8AAAAgxdbUPwAAAEDha+W/AAAAQNZyzT8AAADA7iHq
PwAAAABZG+Y/AAAAAEi/1b8AAAAgVrDYvwAAAGC8oeM/AAAAgKM3fT8AAADgjq/VPwAAAEAIYu6/AAAAoFHN0j8AAABAFK+1PwAA
AKBrLcO/AAAAgHIKlz8AAADA83ruPwAAAEBkHMc/AAAAQP+iuz8AAACgIl3lvwAAAEBIWfW/AAAAoEg24D8AAABAD5DmvwAAAICR
8Lk/AAAAADnllr8AAACgAJrSvwAAAKBcgre/AAAAgMwU6D8AAAAgkqCqPwAAAEAjKr8/AAAAQF5h1L8AAAAgcib1PwAAAMAFHuM/
AAAAQF6b5D8AAABAI+3VPwAAAIBMt74/AAAA4AdRxD8AAABAVvnmPwAAAOCJStY/AAAAoM22xz8AAACgerzlvwAAAAB3Ooq/AAAA
oFWypT8AAACAkq3DPwAAAODLEdE/AAAAAMRs0j8AAADAFMXLvwAAAEDbuu2/AAAAoONnvj8AAABAJpSOvwAAAKBz8NI/AAAAAH7g
2j8AAACAmsXhPwAAAGCb8se/AAAAYIZf3j8AAAAAWEDqPwAAAOCGC6k/AAAAwBBN0j8AAADACEDlPwAAAGBDGd+/AAAAwAfY4L8A
AACgeYLxPwAAAICzEMG/AAAAICyI5r8AAADA3rv5PwAAAGDgAtK/AAAA4LpZ4r8AAABgj7ChvwAAAOCy0ti/AAAAQGyZ5r8AAABg
QX/RPwAAAKBhTcM/AAAAQG8S478AAABACtq8PwAAAOANg+I/AAAAAO96+T8AAACArrzlPwAAAIC+5PK/AAAAQG1Azr8AAACAkbCi
PwAAAEBzXuY/AAAA4Pkdrz8AAADgRKLMvwAAAAA7T8m/AAAAQM/C178AAADgrXfJvwAAAADkWbI/AAAAwMVS4r8AAAAA3GfWvwAA
AECtbde/AAAAYDdC2T8AAAAAYo7ivwAAAODOOui/AAAA4M31xj8AAACgxD+4vwAAACBvQtW/AAAAYIa/6r8AAABgqu2RvwAAAIBl
y/I/AAAAoL8Oyb8AAABgzknfvwAAAGB0D6s/AAAAYNkwkz8AAACghJPoPwAAAEAvFtI/AAAAoEz13D8AAABAPQLbPwAAACDeLbS/
AAAA4O/p+L8AAACA6QTyvwAAAED5JcU/AAAAoP6L0z8AAABgdP9XPwAAAKARV8e/AAAAgDaj6L8AAADAeTjdvwAAAABgQNS/AAAA
AGOR4b8AAADABMjDvwAAAAAowPi/AAAAoEGt0L8AAAAgRU3LvwAAAGCA7+C/AAAAAB5H2L8AAAAg6J/evwAAAOAxENg/AAAAQIOU
vL8AAAAgmSnivwAAAEBeHrg/AAAAgOpb0r8AAADASHCsvwAAAACKJs2/AAAAgGp54j8AAAAA3b3DvwAAAMBZXEe/AAAAQFR11r8A
AACAzuKgPwAAAOCcTOc/AAAAwJVtsz8AAABg7iiqvwAAAMCVEdE/AAAAIOtXlb8AAABgNh/avwAAAEAXUOC/AAAA4FG15L8AAAAg
60eivwAAAMCEVto/AAAAYFLJlr8AAACgHCPwPwAAAKBxsOg/AAAAYKygzj8AAADAH4zGvwAAACBmbvm/AAAAADI4zz8AAABg9RXh
PwAAAKAeiNc/AAAAYP6fwT8AAABgbOfTvwAAACAkSsU/AAAAYD527r8AAABgxBLvvwAAACBxttG/AAAAgNkvxT8AAACgxQb6vwAA
AICUDNs/AAAAwIM9xz8AAADgLjXiPwAAAEDbas+/AAAAYCUQ1L8AAACghOfAvwAAAOBB07c/AAAAwNAs2D8AAADgjBqjvwAAAGDZ
kum/AAAAYP0rwb8AAACg2zXHPwAAAOD3MX2/AAAAwHcypD8AAABgbZjePwAAAEBJb78/AAAA4Ky6zj8AAABACfHaPwAAAGAOpeC/
AAAAgAnHyT8AAABA66vWvwAAAECJtLI/AAAAAJWywL8AAABA9mXzvwAAAKD9H+6/AAAAgJ0S0j8AAABA2ZvZvwAAAABlAJQ/AAAA
wKY64T8AAAAgRSvQPwAAAACo6ts/AAAAoIBK7L8AAACgUCD+vwAAAKATcrU/AAAAwB0u8L8AAAAAJCXqvwAAAIBCbt0/AAAAIKXe
4L8AAABANdC1vwAAAGBGy9g/AAAAoNoDxT8AAAAAeK7fPwAAAGAYIua/AAAAgJsM/j8AAABAOVXYvwAAAEC/6cK/AAAAQMeT5L8A
AADAzirBvwAAAGA0m6k/AAAAwDXc478AAACAd9LevwAAACB6O9g/AAAAQOey8b8AAABAsSu6PwAAAGAIPZy/AAAAYA+fuz8AAACg
wnCzvwAAACCzI+E/AAAAwG4KnL8AAACg+vnyvwAAAIBVn8s/AAAAQJP0pD8AAACAH6LhPwAAAOCREtG/AAAAgOL9xT8AAADAYi22
vwAAAADJgPI/AAAAwHsd6z8AAACAYtjhPwAAACA/P9s/AAAAoLEF+D8AAADgNBjhvwAAACCPFdq/AAAAoDMWcT8AAADgaCHjPwAA
ACAtf8G/AAAAQDKGAEAAAADge4XovwAAAEBblN+/AAAAgHQe0L8AAAAgEmDMvwAAAEAhM/O/AAAAwCKf3z8AAADAsK/cPwAAAADc
xYm/AAAAwGea4L8AAAAgsj/DPwAAAIDCbgBAAAAAAATR9L8AAACgvTnYPwAAAICb38g/AAAAYEgNyb8AAADgLpm3PwAAAODNSt2/
AAAA4Dnqy78AAABgb2rkPwAAAADV9/I/AAAAoFGq1T8AAACgs7asPwAAAIA6UKW/AAAAAIsUxD8AAADAQdLmPwAAAMDJnNG/AAAA
ICnNt78AAADABcflPwAAAGCr754/AAAAwJXC6z8AAADgH1G2vwAAAGARP/a/AAAAgA9Pvb8AAACgcTrkvwAAAEB3Y8W/AAAAgDXO
8z8AAABgajPkvwAAAKBIM+m/AAAAANPO6D8AAACAOx7APwAAAAA52r+/AAAAQGFD4b8AAADAXdzwPwAAAAAGTwDAAAAAIKQU4b8A
AABg6OTIvwAAAODaBfA/AAAAgCVb2r8AAAAAw/3hvwAAAKCqAeM/AAAAwOnRij8AAADAoCLMvwAAAKBWaea/AAAA4P+q7T8AAACA
OBUAQAAAAECR/ty/AAAAoGzW5b8AAABgYS6wvwAAAOBvJKa/AAAAwN6H0D8AAAAA56zhvwAAAMDFM9a/AAAAwBsT7r8AAAAARH/Q
vwAAAOBbjLe/AAAAwHpZkL8AAACATezjvwAAAECzEHQ/AAAAQMvO5b8AAABA7VJgvwAAAGCtpuG/AAAAIG5Byb8AAAAgLjisvwAA
AOAhb8E/AAAAIN/o4j8AAACgR33iPwAAAMDPE4i/AAAAALb/zD8AAAAgUa3UPwAAAMAJR8C/AAAAAIAZ4r8AAAAAF4u7PwAAACAa
ruI/AAAAABfXzr8AAACAzfzyvwAAAKCUeN2/AAAAgL/57L8AAABA7kv7PwAAAEB7uNi/AAAAINzb0r8AAADg637OPwAAAIAzQNs/
AAAAgD/Hp78AAABgWzXYvwAAAMDUusQ/AAAAwMHP8T8AAADA4SjcPwAAAKCMA+k/AAAAwESr/D8AAACgSeiqvwAAAMBhpt0/AAAA
IGnxzz8AAACAXmPcPwAAACD+XKC/AAAAIOwc6D8AAAAAc+rePwAAAEALMqG/AAAA4F05ub8AAADAAhPIPwAAAAAp94i/AAAAYFG5
1r8AAAAAtHSTvwAAAEBUo+G/AAAA4ErdhD8AAADg7YrkPwAAAKARbMw/AAAAoCG9vr8AAADgnoyuPwAAAEAXH7M/AAAAIM5VoD8A
AACgMD2HPw==
"""


def _decode_v():
    b = base64.b64decode("".join(_V64_B64.split()))
    return np.frombuffer(b, dtype=np.float64).reshape(64, 11).copy()


# ----------------------------------------------------------------------------
# host-side math prep (small params + O(n) packing; no O(n) flops beyond
# gathers/sorts)
# ----------------------------------------------------------------------------

def _fit_dirs(P_t):
    """Fit 62 directions V (62,11) and Lam (62,32) such that
    sum_m Lam[m,c] (v_m . x)^2 ~= x^T P_t[c] x with ALL entries constrained
    (including (10,10)).  Warm-started from the stored 64-dir fit."""
    V0 = _decode_v()
    Pf = P_t.reshape(N_C, -1)                     # (32,121)
    scale = np.linalg.norm(Pf)

    def lam_and_res(V):
        B = np.einsum('mi,mj->mij', V, V).reshape(len(V), -1)
        Lam, *_ = np.linalg.lstsq(B.T, Pf.T, rcond=None)
        R = B.T @ Lam - Pf.T                      # (121,32)
        return Lam, R, B

    Lam64, _, _ = lam_and_res(V0)
    keep = np.argsort(-np.abs(Lam64).sum(axis=1))[:M_DIRS]
    V = V0[keep].copy()

    Lam, R, B = lam_and_res(V)
    m = np.zeros_like(V); v2 = np.zeros_like(V)
    lr, b1, b2, eps = 3e-3, 0.9, 0.999, 1e-8
    best = (np.inf, V.copy(), Lam)
    for it in range(8000):
        Lam, R, B = lam_and_res(V)
        loss = float(np.sum(R * R))
        if loss < best[0]:
            best = (loss, V.copy(), Lam)
        if np.sqrt(loss) / scale < 2e-6:
            break
        Rc = (R.T).reshape(N_C, 11, 11)
        G = np.einsum('mc,cij,mj->mi', Lam, Rc + Rc.transpose(0, 2, 1), V)
        m = b1 * m + (1 - b1) * G
        v2 = b2 * v2 + (1 - b2) * G * G
        mh = m / (1 - b1 ** (it + 1)); vh = v2 / (1 - b2 ** (it + 1))
        V = V - lr * mh / (np.sqrt(vh) + eps)
    loss, V, Lam = best
    return V, Lam, np.sqrt(loss) / scale


def _host_prep(s, y, ks, ts, means, covs, b_mu, b_log_sig, beta_mu, beta_log_sig):
    f8 = np.float64
    means8, covs8 = means.astype(f8), covs.astype(f8)
    P = np.linalg.inv(covs8)
    P = 0.5 * (P + P.transpose(0, 2, 1))
    sign, logdet = np.linalg.slogdet(covs8)
    assert np.all(sign > 0)

    # P_tilde (32,11,11): s~^T Pt s~ = ll[c](s) + b_c  (constants in (10,10))
    w = np.einsum('cij,cj->ci', P, means8)
    muPmu = np.einsum('ci,cij,cj->c', means8, P, means8)
    Kc = -0.5 * muPmu - 0.5 * logdet - 0.5 * N_D * LOG2PI
    Pt = np.zeros((N_C, 11, 11))
    Pt[:, :10, :10] = -0.5 * P
    Pt[:, :10, 10] = 0.5 * w
    Pt[:, 10, :10] = 0.5 * w
    Pt1010 = Kc + b_mu.astype(f8)

    # overflow shift: device logits A <= max_c(Pt1010_c + g*beta_ct); keep
    # exp(A) well inside f16 range by subtracting S from every class const.
    ks64 = ks.astype(np.int64); ts64 = ts.astype(np.int64)
    g_all = y[ks64, ts64].astype(f8)                          # (n,)
    beta8 = beta_mu.astype(f8)                                # (32,128)
    bound = (Pt1010[None, :] + g_all[:, None] * beta8[:, ts64].T).max(axis=1)
    S = float(max(0.0, bound.max() - 80.0))
    Pt[:, 10, 10] = Pt1010 - S

    V, Lam, res = _fit_dirs(Pt)

    # per-(k,t) LSE table L (f64) and its sum over spikes (host-only term)
    y8 = y.astype(f8)
    ll_kct = b_mu.astype(f8)[None, :, None] + \
        beta_mu.astype(f8)[None, :, :] * y8[:, None, :]      # (256,32,128)
    mx = ll_kct.max(axis=1)
    L = mx + np.log(np.exp(ll_kct - mx[:, None, :]).sum(axis=1))  # (256,128)
    L_sum = float(L[ks64, ts64].sum())

    # prior - q const (f64, formulas of the reference)
    lp = -0.5 * (b_mu.astype(f8) ** 2 + LOG2PI).sum() \
         - 0.5 * (beta_mu.astype(f8) ** 2 + LOG2PI).sum()
    lq = (-0.5 * LOG2PI * b_mu.size - b_log_sig.astype(f8).sum()) + \
         (-0.5 * LOG2PI * beta_mu.size - beta_log_sig.astype(f8).sum())
    elbo_const = lp - lq

    # --- bucket spikes by t; static window->t map shared by all cores ---
    order = np.argsort(ts64, kind='stable')
    counts = np.bincount(ts64, minlength=N_T)                 # (128,)
    w_t = np.maximum(1, -(-counts // (N_CORES * 2 * WIN)))    # windows/core/t
    T_wins = np.repeat(np.arange(N_T), w_t)                   # per-core windows
    n_win = len(T_wins)
    n_win_pad = -(-n_win // 16) * 16
    C = n_win_pad // 16
    T_wins = np.concatenate([T_wins, np.zeros(n_win_pad - n_win, np.int64)])
    NLOC = C * CHUNK                                          # spikes per core

    # window start offsets per bucket (in spikes, per core)
    starts = np.concatenate([[0], np.cumsum(w_t) * 2 * WIN])  # (129,)

    s_aug = np.zeros((N_SPK, 12), dtype=np.float32)
    s_aug[:, :10] = s
    s_aug[:, 10] = 1.0
    s_aug[:, 11] = g_all

    sp_cores = []
    n_real = np.zeros(N_CORES, np.int64)
    pos = 0
    rec = [np.zeros((NLOC, 12), dtype=np.float32) for _ in range(N_CORES)]
    for t in range(N_T):
        idx_t = order[pos:pos + counts[t]]
        pos += counts[t]
        splits = np.array_split(idx_t, N_CORES)
        for i in range(N_CORES):
            k = len(splits[i])
            rec[i][starts[t]:starts[t] + k] = s_aug[splits[i]]
            n_real[i] += k
    for i in range(N_CORES):
        # pack pairs: (npair, 2, 12) -> (24, npair)
        spk = rec[i].reshape(NLOC // 2, 2, 12).transpose(1, 2, 0).reshape(24, NLOC // 2)
        sp_cores.append(spk.astype(F16))
    pads_total = int(N_CORES * NLOC - n_real.sum())

    # W stationary (24, 128) bf16: per spike 62 quad dirs + 2 g-dirs
    W = np.zeros((24, 128), dtype=np.float32)
    for h in range(2):                                        # A/B half
        r0, c0 = 12 * h, 64 * h
        W[r0:r0 + 11, c0:c0 + M_DIRS] = V.T.astype(np.float32)
        W[r0 + 10, c0 + 62] = 0.5; W[r0 + 11, c0 + 62] = 0.5   # (1+g)/2
        W[r0 + 10, c0 + 63] = -0.5; W[r0 + 11, c0 + 63] = 0.5  # (g-1)/2

    # lam table (128, 128*64) bf16: per t a (128,64) block, block-diag A|B
    lam_all = np.zeros((128, N_T * 64), dtype=np.float32)
    Lam32 = Lam.astype(np.float32)                            # (62,32)
    beta32 = beta_mu.astype(np.float32)                       # (32,128)
    for t in range(N_T):
        blk = lam_all[:, 64 * t:64 * (t + 1)]
        blk[0:M_DIRS, 0:32] = Lam32
        blk[62, 0:32] = beta32[:, t]
        blk[63, 0:32] = -beta32[:, t]
        blk[64:64 + M_DIRS, 32:64] = Lam32
        blk[126, 32:64] = beta32[:, t]
        blk[127, 32:64] = -beta32[:, t]

    return dict(W=W.astype(F16), lam_all=lam_all.astype(F16),
                sp_cores=sp_cores, T_wins=tuple(int(t) for t in T_wins),
                C=C, pads_total=pads_total, L_sum=L_sum, S=S,
                n_real=int(n_real.sum()), elbo_const=elbo_const, fit_res=res)


# ----------------------------------------------------------------------------
# device graph
# ----------------------------------------------------------------------------

_GRAPHS = {}


def _build_graph(C, T_wins):
    key = (C, T_wins)
    if key in _GRAPHS:
        return _GRAPHS[key]

    import concourse.bacc as bacc
    import concourse.mybir as mybir

    dt = mybir.dt
    AF = mybir.ActivationFunctionType
    ALU = mybir.AluOpType
    AX = mybir.AxisListType

    nc = bacc.Bacc("TRN2")
    stk = ExitStack()

    NPAIR = C * CHUNK // 2
    sp_d = nc.declare_dram_parameter("sp", [24, NPAIR], dt.float16, isOutput=False)
    w_d = nc.declare_dram_parameter("wmat", [24, 128], dt.float16, isOutput=False)
    lam_d = nc.declare_dram_parameter("lam", [128, N_T * 64], dt.float16,
                                      isOutput=False)
    out_d = nc.declare_dram_parameter("out", [128, 1], dt.float32, isOutput=True)

    sb = lambda name, shape, d: stk.enter_context(nc.sbuf_tensor(name, shape, d))
    ps = lambda name, shape: stk.enter_context(nc.psum_tensor(name, shape, dt.float32))
    sem = lambda name: stk.enter_context(nc.semaphore(name))

    PCH = CHUNK // 2                  # 2048 pairs per chunk
    spt = [sb(f"spt{i}", [24, PCH], dt.float16) for i in range(3)]
    phi = [sb(f"phi{i}", [128, PCH], dt.float16) for i in range(3)]
    ucp = [sb(f"ucp{i}", [128, 1024], dt.float16) for i in range(2)]
    E = [sb(f"E{i}", [128, 32, 32], dt.float32) for i in range(2)]
    contrib = sb("contrib", [128, C * 32], dt.float32)
    lncon = sb("lncon", [128, C * 32], dt.float32)
    w_sb = sb("w_sb", [24, 128], dt.float16)
    lam_sb = sb("lam_sb", [128, N_T * 64], dt.float16)
    acc_sb = sb("acc_sb", [128, 1], dt.float32)

    Ua = ps("Ua", [128, 1024])
    Ub = ps("Ub", [128, 1024])
    llv = [ps(f"llv{i}", [128, 1024]) for i in range(2)]

    s_lsp = [sem(f"s_lsp{i}") for i in range(3)]
    s_inw = sem("s_inw"); s_inl = sem("s_inl")
    s_mm1 = sem("s_mm1"); s_mmf = sem("s_mmf")
    s_sqa = sem("s_sqa"); s_cp = sem("s_cp"); s_ttq = sem("s_ttq")
    s_exp = sem("s_exp"); s_red = sem("s_red")
    s_log = sem("s_log"); s_acc = sem("s_acc"); s_out = sem("s_out")

    with nc.Block() as block:

        @block.sync
        def _(e):
            e.dma_start(out=w_sb[:], in_=w_d[:]).then_inc(s_inw, 16)
            e.dma_start(out=lam_sb[:], in_=lam_d[:]).then_inc(s_inl, 16)
            for c in range(C):
                if c >= 3:
                    e.wait_ge(s_mm1, 4 * (c - 3) + 4)         # spt buf reuse
                e.dma_start(out=spt[c % 3][:], in_=sp_d[:, c * PCH:(c + 1) * PCH]
                            ).then_inc(s_lsp[c % 3], 16)
            e.wait_ge(s_acc, 1)
            e.dma_start(out=out_d[:], in_=acc_sb[:]).then_inc(s_out, 16)
            e.wait_ge(s_out, 16)

        @block.tensor
        def _(e):
            e.wait_ge(s_inw, 16)
            for g in range(C + 2):
                cm = g - 2
                if cm == 0:
                    e.wait_ge(s_inl, 16)                       # lam table loaded
                if 0 <= cm < C:
                    if cm >= 2:
                        e.wait_ge(s_exp, cm - 1)               # llv buf reuse
                    for wd in range(16):
                        if wd == 0:
                            e.wait_ge(s_sqa, cm + 1)           # phi half A
                        if wd == 8:
                            e.wait_ge(s_ttq, cm + 1)           # phi half B
                        t = T_wins[16 * cm + wd]
                        e.matmul(llv[cm % 2][:, 64 * wd:64 * wd + 64],
                                 phi[cm % 3][:, 128 * wd:128 * (wd + 1)],
                                 lam_sb[:, 64 * t:64 * t + 64],
                                 start=True, stop=True).then_inc(s_mmf, 1)
                if g < C:
                    c = g
                    e.wait_ge(s_lsp[c % 3], 16 * (c // 3 + 1))
                    for j in range(4):
                        if c >= 1:
                            if j == 0:
                                e.wait_ge(s_sqa, c)            # Ua reuse
                            if j == 2:
                                e.wait_ge(s_cp, c)             # Ub reuse
                        U = Ua if j < 2 else Ub
                        e.matmul(U[:, (j % 2) * 512:(j % 2 + 1) * 512],
                                 w_sb[:], spt[c % 3][:, j * 512:(j + 1) * 512],
                                 start=True, stop=True).then_inc(s_mm1, 1)

        @block.scalar
        def _(e):
            for g in range(C + 3):
                c1 = g - 1
                if 0 <= c1 < C:                                # square half A
                    e.wait_ge(s_mm1, 4 * c1 + 2)
                    if c1 >= 3:                                # phi buf reuse
                        e.wait_ge(s_mmf, 16 * (c1 - 2))
                    e.activation(phi[c1 % 3][:, 0:1024], Ua[:],
                                 AF.Square).then_inc(s_sqa, 1)
                c2 = g - 3
                if 0 <= c2 < C:                                # exp
                    e.wait_ge(s_mmf, 16 * c2 + 16)
                    if c2 >= 2:                                # E buf reuse
                        e.wait_ge(s_red, c2 - 1)
                    e.activation(E[c2 % 2][:], llv[c2 % 2][:],
                                 AF.Exp).then_inc(s_exp, 1)
            e.wait_ge(s_red, C)
            e.activation(lncon[:], contrib[:], AF.Ln).then_inc(s_log, 1)

        @block.vector
        def _(e):
            for g in range(C + 4):
                c1 = g - 1
                if 0 <= c1 < C:                                # copy Ub -> f16
                    e.wait_ge(s_mm1, 4 * c1 + 4)
                    if c1 >= 2:                                # ucp buf reuse
                        e.wait_ge(s_ttq, c1 - 1)
                    e.tensor_copy(ucp[c1 % 2][:], Ub[:]).then_inc(s_cp, 1)
                cr = g - 4
                if 0 <= cr < C:                                # segmented reduce
                    e.wait_ge(s_exp, cr + 1)
                    e.tensor_reduce(contrib[:, 32 * cr:32 * cr + 32],
                                    E[cr % 2][:], AX.X, ALU.add
                                    ).then_inc(s_red, 1)
            e.wait_ge(s_log, 1)
            e.tensor_reduce(acc_sb[:], lncon[:], AX.X, ALU.add).then_inc(s_acc, 1)

        @block.gpsimd
        def _(e):
            for c1 in range(C):                                # square half B
                e.wait_ge(s_cp, c1 + 1)
                if c1 >= 3:                                    # phi buf reuse
                    e.wait_ge(s_mmf, 16 * (c1 - 2))
                e.tensor_tensor(phi[c1 % 3][:, 1024:2048],
                                ucp[c1 % 2][:], ucp[c1 % 2][:],
                                ALU.mult).then_inc(s_ttq, 1)

    nc.compile()
    _GRAPHS[key] = nc
    return nc


# ----------------------------------------------------------------------------
# entry point
# ----------------------------------------------------------------------------

LAST_RESULTS = None


def kernel(s, y, ks, ts, means, covs, b_mu, b_log_sig, beta_mu, beta_log_sig):
    import os
    global LAST_RESULTS
    s = np.asarray(s); y = np.asarray(y)
    ks = np.asarray(ks); ts = np.asarray(ts)
    means = np.asarray(means); covs = np.asarray(covs)
    b_mu = np.asarray(b_mu); b_log_sig = np.asarray(b_log_sig)
    beta_mu = np.asarray(beta_mu); beta_log_sig = np.asarray(beta_log_sig)

    prep = _host_prep(s, y, ks, ts, means, covs, b_mu,
                      b_log_sig, beta_mu, beta_log_sig)

    nc = _build_graph(prep["C"], prep["T_wins"])
    from concourse.bass_utils import run_bass_kernel_spmd

    in_maps = []
    for i in range(N_CORES):
        in_maps.append({
            "sp": np.asarray(prep["sp_cores"][i]),
            "wmat": np.asarray(prep["W"]),
            "lam": np.asarray(prep["lam_all"]),
        })

    trace = bool(os.environ.get("BASS_TRACE"))
    res = run_bass_kernel_spmd(nc, in_maps, core_ids=list(range(N_CORES)),
                               trace=trace)
    LAST_RESULTS = res

    partials = [float(res.results[i]["out"].astype(np.float64).sum())
                for i in range(N_CORES)]
    total = (sum(partials)
             + prep["S"] * prep["n_real"]
             - prep["pads_total"] * float(np.log(32.0))
             - prep["L_sum"]
             + prep["elbo_const"])
    return np.float32(total)


# revision 14
# speedup vs baseline: 8.2730x; 1.0920x over previous
"""Trainium2 Bass kernel for nn_ADVI (segment_reduce ELBO).

Math:
  elbo = const(prior - q) + sum_n LSE_c( ll[n,c] + log_pis[ks_n, c, ts_n] )
  log_pis[k,c,t] = b_c + beta[c,t]*y[k,t] - L[k,t]   (L = LSE_c of the first part)
  The -L[k,t] term is class-independent -> sum_n L[ks_n,ts_n] is computed on host.
  Remaining device math per spike:  A[n,c] = s~^T Pt_c s~ + g_n * beta[c, t_n]
  with s~ = [s;1], g_n = y[ks_n, ts_n], and Pt_c carrying b_c + all constants in
  its (10,10) entry.  The quadratic is fit EXACTLY (res ~2e-6) as
  sum_m lam[m,c] (v_m . s~)^2 over 62 shared directions; two extra exact
  "directions" ((g+1)/2)^2 and ((g-1)/2)^2 with coefficients +-beta[c,t]
  reconstruct g*beta.  Spikes are host-sorted into 128 t-buckets so each
  128-pair matmul window uses one lam_t; the window->t map is static and
  identical on all 8 cores (per-bucket window counts are globally padded).

  Device pipeline per chunk (4096 spikes = 2048 pair-columns, 2 spikes/col):
  DMA sp -> PE mm1 (proj to 128 dirs) -> square (ACT half / DVE half, ->bf16)
  -> PE mm2 vs lam_t table (out [128 pairs, 64] = A|B classes) -> ACT exp
  -> DVE segmented reduce (sum 32 classes) -> contrib.  One deferred Ln over
  all contribs at the end (avoids ACT table-set thrash), then reduce+matmul
  to a scalar.  No gather, no gpsimd work.
"""

import base64
import sys
from contextlib import ExitStack

import numpy as np

sys.path.insert(0, "/opt/trn_rl_repo")

import ml_dtypes  # noqa: E402

F16 = np.float16

LOG2PI = float(np.log(2.0 * np.pi))
N_K, N_T, N_C, N_D, N_SPK = 256, 128, 32, 10, 1000000
N_CORES = 8
CHUNK = 4096                      # spikes per chunk
WIN = 128                         # pairs per mm2 window (256 spikes)
M_DIRS = 62                       # fitted quadratic directions per spike

# 64 fitted directions (f64, 64x11) from the original reference fit; used as
# warm start for the 62-dir constrained refinement.
_V64_B64 = """
AAAAoH/q7z8AAADAZxKMPwAAAGC3gpK/AAAAYMKUkD8AAABAftuSvwAAAMD4rJq/AAAAIPyonj8AAABgqPmTPwAAAEAhMZI/AAAA
wOuImD8AAADgfAYIvwAAAAA0dIq/AAAAAG1L8D8AAABgiy+LvwAAAOChXpI/AAAAIJoyVj8AAACAp5SfPwAAACBKWni/AAAAQP9B
Qj8AAACA7E9svwAAAEAbVoO/AAAAAMFbk78AAABgyQqWPwAAAACRppK/AAAAgNFA7z8AAADgRMSUvwAAACAXYqE/AAAA4C9omL8A
AABgzk+MvwAAAACFkaE/AAAAgBYBkD8AAABAwQmbPwAAAMDHDY4/AAAAIAfUeL8AAACgFpiZvwAAAMB6ezK/AAAAoFOO8D8AAAAA
fSGRPwAAAMDzCpo/AAAAgFvYob8AAACAdd9zvwAAAAAHsYy/AAAAYDYjdj8AAAAgq6ybvwAAAGCAyZi/AAAAwORZeL8AAAAgkaqb
PwAAAGBoKJc/AAAAADpY7z8AAAAgFQabPwAAAGCR5Za/AAAAoN5dcT8AAADA7yyiPwAAACBJGWm/AAAAgMEakD8AAAAg5MGHPwAA
ACDI6om/AAAA4Bu6mr8AAAAghwSgvwAAAEBQuJ2/AAAAAIlT8D8AAACAoh2NvwAAAMDATKe/AAAAIPH4lj8AAACAN3JRPwAAAEA+
ipG/AAAA4MOIoj8AAADAWhqLvwAAAOAoZpA/AAAAIBbdkz8AAAAAv2GXvwAAAECqzJu/AAAAgDgG8D8AAAAAlYlaPwAAAGD6cIA/
AAAAgEdPmD8AAACAFWt9vwAAAGC7wZM/AAAAIG9KlL8AAABAXB2gPwAAACCKw5Q/AAAAwNc1hb8AAADAuhebPwAAAKAdFYg/AAAA
oImF8D8AAAAgVhyRvwAAAKAOMUU/AAAAoJFBq78AAADgFmd5PwAAAACdT5Q/AAAAwBoOlL8AAADAOPyOvwAAAOAhHpo/AAAAQJLF
k78AAADgQe+VPwAAAOBWfJq/AAAAgLgi8D8AAADA2yyCPwAAAOC96Y2/AAAAoCRGob8AAACASjBtPwAAACD7GIY/AAAAoLxHdj8A
AADA9/mDPwAAAGAT/YQ/AAAAQKxiZr8AAABAolWSvwAAAECaAnu/AAAAAJx/8D8AAABAzH2cvwAAACBK9+8/AAAAYAyajb8AAAAA
X8SUPwAAAGAMKZC/AAAAoF+YlD8AAADANBCYPwAAACCmSZK/AAAAwNllj78AAAAgHyeMvwAAAGBB1Zq/AAAAwHEo8D8AAACAiuyO
PwAAAGD4fO8/AAAAwJRTiz8AAADA1XWLvwAAAICtNHG/AAAA4Nbxn78AAADAWD1/PwAAAECEtD8/AAAAQPTfRT8AAABg852GPwAA
AIDISPA/AAAAwBRfkb8AAAAgOxJzPwAAAID6zO4/AAAAoJ/pfD8AAACA2zJ+vwAAAADpEoQ/AAAAAJQ7oj8AAAAAToeTvwAAAMDi
bYi/AAAAINCVj78AAACgrofwPwAAAMBaQ4w/AAAAoDJLnj8AAABAIxN8PwAAAOBBo/A/AAAAYCNFib8AAACgHnycvwAAAEDB5Kc/
AAAAIJG/jD8AAADgvKuBPwAAAABjioW/AAAAIId67j8AAABA91GXPwAAAEBYZU4/AAAAQHxIn78AAABgvBOWvwAAAIABS/A/AAAA
gNsAmL8AAABA1syXPwAAAOBNSW6/AAAAQEUUob8AAAAggTxivwAAACBQje8/AAAAoNnyhb8AAABg1N6DPwAAAGD3bmm/AAAAYCZT
mj8AAACg1kqdPwAAACBTo+4/AAAAAE4fij8AAADAnm+gPwAAAACXX4m/AAAA4HqUir8AAACAip7wPwAAAOCFKqK/AAAAIC0Qaz8A
AACgwpSavwAAAOC8aIy/AAAAACfalj8AAACg4viTPwAAAMBAte8/AAAAwHGml78AAABg6zaNvwAAAODNapC/AAAAgGk48D8AAADg
TGqcvwAAAID7tJE/AAAAYMYOob8AAAAg7A2RvwAAAIC4fWy/AAAAgNxWib8AAABgAteRvwAAACA2X+4/AAAAAKAnoD8AAAAAwJSG
vwAAAADtFPE/AAAAAA70hb8AAABAA9CTvwAAAKAxrpQ/AAAAoPUEkD8AAADAjSKYvwAAAIBnKJU/AAAAINyOmL8AAABA3Q6gPwAA
AKAsYPA/AAAAIOqql78AAACAPYzvPwAAAKAI3Zw/AAAAAOydYb8AAADAK5yAvwAAAEAQt32/AAAAoNLdhL8AAABg8g6UvwAAAOCo
cXI/AAAAIM91iz8AAADA9oyCPwAAAAAzT+8/AAAAgFRm8D8AAAAAf3PrPwAAAOBKGeI/AAAAgIl40r8AAADAE/TgvwAAAICSDrW/
AAAAIBOK6T8AAABgUrbMPwAAAIBGiuY/AAAAAIjZyD8AAACgLbjOPwAAAMB3EvI/AAAAYAxvor8AAACgqNzuPwAAAICA9ea/AAAA
4NskwT8AAAAAddHCPwAAACANYem/AAAAQMSt4L8AAAAgwJTHPwAAAIDdOqa/AAAAIIMe1j8AAADgOZmwvwAAAIAOZeE/AAAAgJKm
wj8AAACAZgDQvwAAAEDoc7m/AAAA4PbP1b8AAADA1VfYPwAAAICHW8o/AAAAoLXd3j8AAABAYhLLPwAAAKBdIrI/AAAAwHWv878A
AABgd4HjvwAAAICvlsm/AAAAAJkizr8AAADAI+LWvwAAAKBTx9e/AAAAADWflb8AAABgFzunPwAAAMARQa4/AAAAYIPZ5z8AAACg
mI/lPwAAAAAc66q/AAAAALOHxz8AAABgJ1boPwAAAKC4iNc/AAAAQByC4T8AAABADrHlvwAAAABrcry/AAAAILZ+5b8AAADAaXDw
vwAAAEDQHve/AAAAAEMNxr8AAABgc0f7PwAAAEAQHao/AAAA4Dnj2b8AAADgnjTJvwAAAIB5gNG/AAAAANUv4T8AAADgwC+jPwAA
AIC7adM/AAAAoCda4T8AAABA/CjwPwAAAOBztNA/AAAAAOFp+z8AAADgKvjJPwAAAABcaNw/AAAAgOFDtz8AAADgFR+xvwAAAMD2
ZPC/AAAAoE8h7z8AAABgIRzqPwAAAOAFl+E/AAAA4Gp7rr8AAABgbAHrPwAAAEDb37Y/AAAAIOWg578AAAAAiDruvwAAACD4pOS/
AAAAgKwc0L8AAABgtoHhvwAAAGAxoMS/AAAA4Nh81b8AAAAgfUPhvwAAAIBxJKy/AAAAoIOc7T8AAAAgiha4PwAAAGAu+/Y/AAAA
gIUH0T8AAABA88TWvwAAAMB9V+U/AAAA4PUWu78AAACAgZC+PwAAAEBuItO/AAAAYGIUnj8AAACA3EzyPwAAAGDVDtk/AAAAYP1t
+z8AAAAA7irtPwAAAOBVd7C/AAAAQGTjyL8AAAAgUcnpPwAAAIAnvqm/AAAAIJ8vUz8AAABAgQ65vwAAACB/GHw/AAAAoBK07T8A
AAAgVaDQPwAAAID8Pvu/AAAAQItmZr8AAAAAWuvzvwAAAAAM+MK/AAAA4CjN1j8AAAAAqDHVPwAAAIB858C/AAAAQHXvyD8AAACg
eYXxvwAAACBNCrS/AAAAAFOLxT8AAABgYzSyPwAAAMAM5uU/AAAAQFuAtD8AAAAgxdbUPwAAAEDha+W/AAAAQNZyzT8AAADA7iHq
PwAAAABZG+Y/AAAAAEi/1b8AAAAgVrDYvwAAAGC8oeM/AAAAgKM3fT8AAADgjq/VPwAAAEAIYu6/AAAAoFHN0j8AAABAFK+1PwAA
AKBrLcO/AAAAgHIKlz8AAADA83ruPwAAAEBkHMc/AAAAQP+iuz8AAACgIl3lvwAAAEBIWfW/AAAAoEg24D8AAABAD5DmvwAAAICR
8Lk/AAAAADnllr8AAACgAJrSvwAAAKBcgre/AAAAgMwU6D8AAAAgkqCqPwAAAEAjKr8/AAAAQF5h1L8AAAAgcib1PwAAAMAFHuM/
AAAAQF6b5D8AAABAI+3VPwAAAIBMt74/AAAA4AdRxD8AAABAVvnmPwAAAOCJStY/AAAAoM22xz8AAACgerzlvwAAAAB3Ooq/AAAA
oFWypT8AAACAkq3DPwAAAODLEdE/AAAAAMRs0j8AAADAFMXLvwAAAEDbuu2/AAAAoONnvj8AAABAJpSOvwAAAKBz8NI/AAAAAH7g
2j8AAACAmsXhPwAAAGCb8se/AAAAYIZf3j8AAAAAWEDqPwAAAOCGC6k/AAAAwBBN0j8AAADACEDlPwAAAGBDGd+/AAAAwAfY4L8A
AACgeYLxPwAAAICzEMG/AAAAICyI5r8AAADA3rv5PwAAAGDgAtK/AAAA4LpZ4r8AAABgj7ChvwAAAOCy0ti/AAAAQGyZ5r8AAABg
QX/RPwAAAKBhTcM/AAAAQG8S478AAABACtq8PwAAAOANg+I/AAAAAO96+T8AAACArrzlPwAAAIC+5PK/AAAAQG1Azr8AAACAkbCi
PwAAAEBzXuY/AAAA4Pkdrz8AAADgRKLMvwAAAAA7T8m/AAAAQM/C178AAADgrXfJvwAAAADkWbI/AAAAwMVS4r8AAAAA3GfWvwAA
AECtbde/AAAAYDdC2T8AAAAAYo7ivwAAAODOOui/AAAA4M31xj8AAACgxD+4vwAAACBvQtW/AAAAYIa/6r8AAABgqu2RvwAAAIBl
y/I/AAAAoL8Oyb8AAABgzknfvwAAAGB0D6s/AAAAYNkwkz8AAACghJPoPwAAAEAvFtI/AAAAoEz13D8AAABAPQLbPwAAACDeLbS/
AAAA4O/p+L8AAACA6QTyvwAAAED5JcU/AAAAoP6L0z8AAABgdP9XPwAAAKARV8e/AAAAgDaj6L8AAADAeTjdvwAAAABgQNS/AAAA
AGOR4b8AAADABMjDvwAAAAAowPi/AAAAoEGt0L8AAAAgRU3LvwAAAGCA7+C/AAAAAB5H2L8AAAAg6J/evwAAAOAxENg/AAAAQIOU
vL8AAAAgmSnivwAAAEBeHrg/AAAAgOpb0r8AAADASHCsvwAAAACKJs2/AAAAgGp54j8AAAAA3b3DvwAAAMBZXEe/AAAAQFR11r8A
AACAzuKgPwAAAOCcTOc/AAAAwJVtsz8AAABg7iiqvwAAAMCVEdE/AAAAIOtXlb8AAABgNh/avwAAAEAXUOC/AAAA4FG15L8AAAAg
60eivwAAAMCEVto/AAAAYFLJlr8AAACgHCPwPwAAAKBxsOg/AAAAYKygzj8AAADAH4zGvwAAACBmbvm/AAAAADI4zz8AAABg9RXh
PwAAAKAeiNc/AAAAYP6fwT8AAABgbOfTvwAAACAkSsU/AAAAYD527r8AAABgxBLvvwAAACBxttG/AAAAgNkvxT8AAACgxQb6vwAA
AICUDNs/AAAAwIM9xz8AAADgLjXiPwAAAEDbas+/AAAAYCUQ1L8AAACghOfAvwAAAOBB07c/AAAAwNAs2D8AAADgjBqjvwAAAGDZ
kum/AAAAYP0rwb8AAACg2zXHPwAAAOD3MX2/AAAAwHcypD8AAABgbZjePwAAAEBJb78/AAAA4Ky6zj8AAABACfHaPwAAAGAOpeC/
AAAAgAnHyT8AAABA66vWvwAAAECJtLI/AAAAAJWywL8AAABA9mXzvwAAAKD9H+6/AAAAgJ0S0j8AAABA2ZvZvwAAAABlAJQ/AAAA
wKY64T8AAAAgRSvQPwAAAACo6ts/AAAAoIBK7L8AAACgUCD+vwAAAKATcrU/AAAAwB0u8L8AAAAAJCXqvwAAAIBCbt0/AAAAIKXe
4L8AAABANdC1vwAAAGBGy9g/AAAAoNoDxT8AAAAAeK7fPwAAAGAYIua/AAAAgJsM/j8AAABAOVXYvwAAAEC/6cK/AAAAQMeT5L8A
AADAzirBvwAAAGA0m6k/AAAAwDXc478AAACAd9LevwAAACB6O9g/AAAAQOey8b8AAABAsSu6PwAAAGAIPZy/AAAAYA+fuz8AAACg
wnCzvwAAACCzI+E/AAAAwG4KnL8AAACg+vnyvwAAAIBVn8s/AAAAQJP0pD8AAACAH6LhPwAAAOCREtG/AAAAgOL9xT8AAADAYi22
vwAAAADJgPI/AAAAwHsd6z8AAACAYtjhPwAAACA/P9s/AAAAoLEF+D8AAADgNBjhvwAAACCPFdq/AAAAoDMWcT8AAADgaCHjPwAA
ACAtf8G/AAAAQDKGAEAAAADge4XovwAAAEBblN+/AAAAgHQe0L8AAAAgEmDMvwAAAEAhM/O/AAAAwCKf3z8AAADAsK/cPwAAAADc
xYm/AAAAwGea4L8AAAAgsj/DPwAAAIDCbgBAAAAAAATR9L8AAACgvTnYPwAAAICb38g/AAAAYEgNyb8AAADgLpm3PwAAAODNSt2/
AAAA4Dnqy78AAABgb2rkPwAAAADV9/I/AAAAoFGq1T8AAACgs7asPwAAAIA6UKW/AAAAAIsUxD8AAADAQdLmPwAAAMDJnNG/AAAA
ICnNt78AAADABcflPwAAAGCr754/AAAAwJXC6z8AAADgH1G2vwAAAGARP/a/AAAAgA9Pvb8AAACgcTrkvwAAAEB3Y8W/AAAAgDXO
8z8AAABgajPkvwAAAKBIM+m/AAAAANPO6D8AAACAOx7APwAAAAA52r+/AAAAQGFD4b8AAADAXdzwPwAAAAAGTwDAAAAAIKQU4b8A
AABg6OTIvwAAAODaBfA/AAAAgCVb2r8AAAAAw/3hvwAAAKCqAeM/AAAAwOnRij8AAADAoCLMvwAAAKBWaea/AAAA4P+q7T8AAACA
OBUAQAAAAECR/ty/AAAAoGzW5b8AAABgYS6wvwAAAOBvJKa/AAAAwN6H0D8AAAAA56zhvwAAAMDFM9a/AAAAwBsT7r8AAAAARH/Q
vwAAAOBbjLe/AAAAwHpZkL8AAACATezjvwAAAECzEHQ/AAAAQMvO5b8AAABA7VJgvwAAAGCtpuG/AAAAIG5Byb8AAAAgLjisvwAA
AOAhb8E/AAAAIN/o4j8AAACgR33iPwAAAMDPE4i/AAAAALb/zD8AAAAgUa3UPwAAAMAJR8C/AAAAAIAZ4r8AAAAAF4u7PwAAACAa
ruI/AAAAABfXzr8AAACAzfzyvwAAAKCUeN2/AAAAgL/57L8AAABA7kv7PwAAAEB7uNi/AAAAINzb0r8AAADg637OPwAAAIAzQNs/
AAAAgD/Hp78AAABgWzXYvwAAAMDUusQ/AAAAwMHP8T8AAADA4SjcPwAAAKCMA+k/AAAAwESr/D8AAACgSeiqvwAAAMBhpt0/AAAA
IGnxzz8AAACAXmPcPwAAACD+XKC/AAAAIOwc6D8AAAAAc+rePwAAAEALMqG/AAAA4F05ub8AAADAAhPIPwAAAAAp94i/AAAAYFG5
1r8AAAAAtHSTvwAAAEBUo+G/AAAA4ErdhD8AAADg7YrkPwAAAKARbMw/AAAAoCG9vr8AAADgnoyuPwAAAEAXH7M/AAAAIM5VoD8A
AACgMD2HPw==
"""


def _decode_v():
    b = base64.b64decode("".join(_V64_B64.split()))
    return np.frombuffer(b, dtype=np.float64).reshape(64, 11).copy()


# ----------------------------------------------------------------------------
# host-side math prep (small params + O(n) packing; no O(n) flops beyond
# gathers/sorts)
# ----------------------------------------------------------------------------

def _fit_dirs(P_t):
    """Fit 62 directions V (62,11) and Lam (62,32) such that
    sum_m Lam[m,c] (v_m . x)^2 ~= x^T P_t[c] x with ALL entries constrained
    (including (10,10)).  Warm-started from the stored 64-dir fit."""
    V0 = _decode_v()
    Pf = P_t.reshape(N_C, -1)                     # (32,121)
    scale = np.linalg.norm(Pf)

    def lam_and_res(V):
        B = np.einsum('mi,mj->mij', V, V).reshape(len(V), -1)
        Lam, *_ = np.linalg.lstsq(B.T, Pf.T, rcond=None)
        R = B.T @ Lam - Pf.T                      # (121,32)
        return Lam, R, B

    Lam64, _, _ = lam_and_res(V0)
    keep = np.argsort(-np.abs(Lam64).sum(axis=1))[:M_DIRS]
    V = V0[keep].copy()

    Lam, R, B = lam_and_res(V)
    m = np.zeros_like(V); v2 = np.zeros_like(V)
    lr, b1, b2, eps = 3e-3, 0.9, 0.999, 1e-8
    best = (np.inf, V.copy(), Lam)
    for it in range(8000):
        Lam, R, B = lam_and_res(V)
        loss = float(np.sum(R * R))
        if loss < best[0]:
            best = (loss, V.copy(), Lam)
        if np.sqrt(loss) / scale < 2e-6:
            break
        Rc = (R.T).reshape(N_C, 11, 11)
        G = np.einsum('mc,cij,mj->mi', Lam, Rc + Rc.transpose(0, 2, 1), V)
        m = b1 * m + (1 - b1) * G
        v2 = b2 * v2 + (1 - b2) * G * G
        mh = m / (1 - b1 ** (it + 1)); vh = v2 / (1 - b2 ** (it + 1))
        V = V - lr * mh / (np.sqrt(vh) + eps)
    loss, V, Lam = best
    return V, Lam, np.sqrt(loss) / scale


def _host_prep(s, y, ks, ts, means, covs, b_mu, b_log_sig, beta_mu, beta_log_sig):
    f8 = np.float64
    means8, covs8 = means.astype(f8), covs.astype(f8)
    P = np.linalg.inv(covs8)
    P = 0.5 * (P + P.transpose(0, 2, 1))
    sign, logdet = np.linalg.slogdet(covs8)
    assert np.all(sign > 0)

    # P_tilde (32,11,11): s~^T Pt s~ = ll[c](s) + b_c  (constants in (10,10))
    w = np.einsum('cij,cj->ci', P, means8)
    muPmu = np.einsum('ci,cij,cj->c', means8, P, means8)
    Kc = -0.5 * muPmu - 0.5 * logdet - 0.5 * N_D * LOG2PI
    Pt = np.zeros((N_C, 11, 11))
    Pt[:, :10, :10] = -0.5 * P
    Pt[:, :10, 10] = 0.5 * w
    Pt[:, 10, :10] = 0.5 * w
    Pt1010 = Kc + b_mu.astype(f8)

    # overflow shift: device logits A <= max_c(Pt1010_c + g*beta_ct); keep
    # exp(A) well inside f16 range by subtracting S from every class const.
    ks64 = ks.astype(np.int64); ts64 = ts.astype(np.int64)
    g_all = y[ks64, ts64].astype(f8)                          # (n,)
    beta8 = beta_mu.astype(f8)                                # (32,128)
    bound = (Pt1010[None, :] + g_all[:, None] * beta8[:, ts64].T).max(axis=1)
    S = float(max(0.0, bound.max() - 80.0))
    Pt[:, 10, 10] = Pt1010 - S

    V, Lam, res = _fit_dirs(Pt)

    # per-(k,t) LSE table L (f64) and its sum over spikes (host-only term)
    y8 = y.astype(f8)
    ll_kct = b_mu.astype(f8)[None, :, None] + \
        beta_mu.astype(f8)[None, :, :] * y8[:, None, :]      # (256,32,128)
    mx = ll_kct.max(axis=1)
    L = mx + np.log(np.exp(ll_kct - mx[:, None, :]).sum(axis=1))  # (256,128)
    L_sum = float(L[ks64, ts64].sum())

    # prior - q const (f64, formulas of the reference)
    lp = -0.5 * (b_mu.astype(f8) ** 2 + LOG2PI).sum() \
         - 0.5 * (beta_mu.astype(f8) ** 2 + LOG2PI).sum()
    lq = (-0.5 * LOG2PI * b_mu.size - b_log_sig.astype(f8).sum()) + \
         (-0.5 * LOG2PI * beta_mu.size - beta_log_sig.astype(f8).sum())
    elbo_const = lp - lq

    # --- bucket spikes by t; static window->t map shared by all cores ---
    order = np.argsort(ts64, kind='stable')
    counts = np.bincount(ts64, minlength=N_T)                 # (128,)
    w_t = np.maximum(1, -(-counts // (N_CORES * 2 * WIN)))    # windows/core/t
    T_wins = np.repeat(np.arange(N_T), w_t)                   # per-core windows
    n_win = len(T_wins)
    n_win_pad = -(-n_win // 16) * 16
    C = n_win_pad // 16
    T_wins = np.concatenate([T_wins, np.zeros(n_win_pad - n_win, np.int64)])
    NLOC = C * CHUNK                                          # spikes per core

    # window start offsets per bucket (in spikes, per core)
    starts = np.concatenate([[0], np.cumsum(w_t) * 2 * WIN])  # (129,)

    s_aug = np.zeros((N_SPK, 12), dtype=np.float32)
    s_aug[:, :10] = s
    s_aug[:, 10] = 1.0
    s_aug[:, 11] = g_all

    sp_cores = []
    n_real = np.zeros(N_CORES, np.int64)
    pos = 0
    rec = [np.zeros((NLOC, 12), dtype=np.float32) for _ in range(N_CORES)]
    for t in range(N_T):
        idx_t = order[pos:pos + counts[t]]
        pos += counts[t]
        splits = np.array_split(idx_t, N_CORES)
        for i in range(N_CORES):
            k = len(splits[i])
            rec[i][starts[t]:starts[t] + k] = s_aug[splits[i]]
            n_real[i] += k
    for i in range(N_CORES):
        # pack pairs: (npair, 2, 12) -> (24, npair)
        spk = rec[i].reshape(NLOC // 2, 2, 12).transpose(1, 2, 0).reshape(24, NLOC // 2)
        sp_cores.append(spk.astype(F16))
    pads_total = int(N_CORES * NLOC - n_real.sum())

    # W stationary (24, 128) bf16: per spike 62 quad dirs + 2 g-dirs
    W = np.zeros((24, 128), dtype=np.float32)
    for h in range(2):                                        # A/B half
        r0, c0 = 12 * h, 64 * h
        W[r0:r0 + 11, c0:c0 + M_DIRS] = V.T.astype(np.float32)
        W[r0 + 10, c0 + 62] = 0.5; W[r0 + 11, c0 + 62] = 0.5   # (1+g)/2
        W[r0 + 10, c0 + 63] = -0.5; W[r0 + 11, c0 + 63] = 0.5  # (g-1)/2

    # lam table (128, 128*64) bf16: per t a (128,64) block, block-diag A|B
    lam_all = np.zeros((128, N_T * 64), dtype=np.float32)
    Lam32 = Lam.astype(np.float32)                            # (62,32)
    beta32 = beta_mu.astype(np.float32)                       # (32,128)
    for t in range(N_T):
        blk = lam_all[:, 64 * t:64 * (t + 1)]
        blk[0:M_DIRS, 0:32] = Lam32
        blk[62, 0:32] = beta32[:, t]
        blk[63, 0:32] = -beta32[:, t]
        blk[64:64 + M_DIRS, 32:64] = Lam32
        blk[126, 32:64] = beta32[:, t]
        blk[127, 32:64] = -beta32[:, t]

    return dict(W=W.astype(F16), lam_all=lam_all.astype(F16),
                sp_cores=sp_cores, T_wins=tuple(int(t) for t in T_wins),
                C=C, pads_total=pads_total, L_sum=L_sum, S=S,
                n_real=int(n_real.sum()), elbo_const=elbo_const, fit_res=res)


# ----------------------------------------------------------------------------
# device graph
# ----------------------------------------------------------------------------

_GRAPHS = {}


def _build_graph(C, T_wins):
    key = (C, T_wins)
    if key in _GRAPHS:
        return _GRAPHS[key]

    import concourse.bacc as bacc
    import concourse.mybir as mybir

    dt = mybir.dt
    AF = mybir.ActivationFunctionType
    ALU = mybir.AluOpType
    AX = mybir.AxisListType

    nc = bacc.Bacc("TRN2")
    stk = ExitStack()

    NPAIR = C * CHUNK // 2
    sp_d = nc.declare_dram_parameter("sp", [24, NPAIR], dt.float16, isOutput=False)
    w_d = nc.declare_dram_parameter("wmat", [24, 128], dt.float16, isOutput=False)
    lam_d = nc.declare_dram_parameter("lam", [128, N_T * 64], dt.float16,
                                      isOutput=False)
    out_d = nc.declare_dram_parameter("out", [128, 1], dt.float32, isOutput=True)

    sb = lambda name, shape, d: stk.enter_context(nc.sbuf_tensor(name, shape, d))
    ps = lambda name, shape: stk.enter_context(nc.psum_tensor(name, shape, dt.float32))
    sem = lambda name: stk.enter_context(nc.semaphore(name))

    PCH = CHUNK // 2                  # 2048 pairs per chunk
    spt = [sb(f"spt{i}", [24, PCH], dt.float16) for i in range(3)]
    phi = [sb(f"phi{i}", [128, PCH], dt.float16) for i in range(3)]
    ucp = [sb(f"ucp{i}", [128, 512], dt.float16) for i in range(2)]
    E = [sb(f"E{i}", [128, 32, 32], dt.float32) for i in range(2)]
    contrib = sb("contrib", [128, C * 32], dt.float32)
    lncon = sb("lncon", [128, C * 32], dt.float32)
    w_sb = sb("w_sb", [24, 128], dt.float16)
    lam_sb = sb("lam_sb", [128, N_T * 64], dt.float16)
    acc_sb = sb("acc_sb", [128, 1], dt.float32)

    Ua = ps("Ua", [128, 1536])
    Ub = ps("Ub", [128, 512])
    llv = [ps(f"llv{i}", [128, 1024]) for i in range(2)]

    s_lsp = [sem(f"s_lsp{i}") for i in range(3)]
    s_inw = sem("s_inw"); s_inl = sem("s_inl")
    s_mm1 = sem("s_mm1"); s_mmf = sem("s_mmf")
    s_sqa = sem("s_sqa"); s_cp = sem("s_cp"); s_ttq = sem("s_ttq")
    s_exp = sem("s_exp"); s_red = sem("s_red")
    s_log = sem("s_log"); s_acc = sem("s_acc"); s_out = sem("s_out")

    # lam table arrives in 4 pieces; chunk cm needs buckets up to its max t
    lam_piece = [min(4, T_wins[16 * cm + 15] // 32 + 1) for cm in range(C)]

    with nc.Block() as block:

        @block.sync
        def _(e):
            e.dma_start(out=w_sb[:], in_=w_d[:]).then_inc(s_inw, 16)
            for p in range(4):
                e.dma_start(out=lam_sb[:, 2048 * p:2048 * (p + 1)],
                            in_=lam_d[:, 2048 * p:2048 * (p + 1)]
                            ).then_inc(s_inl, 16)
            for c in range(C):
                if c >= 3:
                    e.wait_ge(s_mm1, 4 * (c - 3) + 4)         # spt buf reuse
                e.dma_start(out=spt[c % 3][:], in_=sp_d[:, c * PCH:(c + 1) * PCH]
                            ).then_inc(s_lsp[c % 3], 16)
            e.wait_ge(s_acc, 1)
            e.dma_start(out=out_d[:], in_=acc_sb[:]).then_inc(s_out, 16)
            e.wait_ge(s_out, 16)

        @block.tensor
        def _(e):
            e.wait_ge(s_inw, 16)
            lam_seen = 0
            for g in range(C + 2):
                cm = g - 2
                if 0 <= cm < C:
                    if lam_piece[cm] > lam_seen:
                        lam_seen = lam_piece[cm]
                        e.wait_ge(s_inl, 16 * lam_seen)        # lam staged load
                    if cm >= 2:
                        e.wait_ge(s_exp, cm - 1)               # llv buf reuse
                    for wd in range(16):
                        if wd == 0:
                            e.wait_ge(s_sqa, cm + 1)           # phi cols 0:1536
                        if wd == 12:
                            e.wait_ge(s_ttq, cm + 1)           # phi cols 1536:
                        t = T_wins[16 * cm + wd]
                        e.matmul(llv[cm % 2][:, 64 * wd:64 * wd + 64],
                                 phi[cm % 3][:, 128 * wd:128 * (wd + 1)],
                                 lam_sb[:, 64 * t:64 * t + 64],
                                 start=True, stop=True).then_inc(s_mmf, 1)
                if g < C:
                    c = g
                    e.wait_ge(s_lsp[c % 3], 16 * (c // 3 + 1))
                    if c >= 1:
                        e.wait_ge(s_sqa, c)                    # Ua reuse
                        e.wait_ge(s_cp, c)                     # Ub reuse
                    for j in range(4):
                        dst = (Ua[:, j * 512:(j + 1) * 512] if j < 3
                               else Ub[:])
                        e.matmul(dst, w_sb[:], spt[c % 3][:, j * 512:(j + 1) * 512],
                                 start=True, stop=True).then_inc(s_mm1, 1)

        @block.scalar
        def _(e):
            for g in range(C + 3):
                c1 = g - 1
                if 0 <= c1 < C:                                # square cols 0:1536
                    e.wait_ge(s_mm1, 4 * c1 + 3)
                    if c1 >= 3:                                # phi buf reuse
                        e.wait_ge(s_mmf, 16 * (c1 - 2))
                    e.activation(phi[c1 % 3][:, 0:1536], Ua[:],
                                 AF.Square).then_inc(s_sqa, 1)
                c2 = g - 3
                if 0 <= c2 < C:                                # exp
                    e.wait_ge(s_mmf, 16 * c2 + 16)
                    if c2 >= 2:                                # E buf reuse
                        e.wait_ge(s_red, c2 - 1)
                    e.activation(E[c2 % 2][:], llv[c2 % 2][:],
                                 AF.Exp).then_inc(s_exp, 1)
            e.wait_ge(s_red, C)
            e.activation(lncon[:], contrib[:], AF.Ln).then_inc(s_log, 1)

        @block.vector
        def _(e):
            for g in range(C + 4):
                c1 = g - 1
                if 0 <= c1 < C:                                # square cols 1536:2048
                    e.wait_ge(s_mm1, 4 * c1 + 4)
                    if c1 >= 2:                                # ucp buf reuse
                        e.wait_ge(s_ttq, c1 - 1)
                    e.tensor_copy(ucp[c1 % 2][:], Ub[:]).then_inc(s_cp, 1)
                    if c1 >= 3:                                # phi buf reuse
                        e.wait_ge(s_mmf, 16 * (c1 - 2))
                    e.tensor_tensor(phi[c1 % 3][:, 1536:2048],
                                    ucp[c1 % 2][:], ucp[c1 % 2][:],
                                    ALU.mult).then_inc(s_ttq, 1)
                cr = g - 4
                if 0 <= cr < C:                                # segmented reduce
                    e.wait_ge(s_exp, cr + 1)
                    e.tensor_reduce(contrib[:, 32 * cr:32 * cr + 32],
                                    E[cr % 2][:], AX.X, ALU.add
                                    ).then_inc(s_red, 1)
            e.wait_ge(s_log, 1)
            e.tensor_reduce(acc_sb[:], lncon[:], AX.X, ALU.add).then_inc(s_acc, 1)

    nc.compile()
    _GRAPHS[key] = nc
    return nc


# ----------------------------------------------------------------------------
# entry point
# ----------------------------------------------------------------------------

LAST_RESULTS = None


def kernel(s, y, ks, ts, means, covs, b_mu, b_log_sig, beta_mu, beta_log_sig):
    import os
    global LAST_RESULTS
    s = np.asarray(s); y = np.asarray(y)
    ks = np.asarray(ks); ts = np.asarray(ts)
    means = np.asarray(means); covs = np.asarray(covs)
    b_mu = np.asarray(b_mu); b_log_sig = np.asarray(b_log_sig)
    beta_mu = np.asarray(beta_mu); beta_log_sig = np.asarray(beta_log_sig)

    prep = _host_prep(s, y, ks, ts, means, covs, b_mu,
                      b_log_sig, beta_mu, beta_log_sig)

    nc = _build_graph(prep["C"], prep["T_wins"])
    from concourse.bass_utils import run_bass_kernel_spmd

    in_maps = []
    for i in range(N_CORES):
        in_maps.append({
            "sp": np.asarray(prep["sp_cores"][i]),
            "wmat": np.asarray(prep["W"]),
            "lam": np.asarray(prep["lam_all"]),
        })

    trace = bool(os.environ.get("BASS_TRACE"))
    res = run_bass_kernel_spmd(nc, in_maps, core_ids=list(range(N_CORES)),
                               trace=trace)
    LAST_RESULTS = res

    partials = [float(res.results[i]["out"].astype(np.float64).sum())
                for i in range(N_CORES)]
    total = (sum(partials)
             + prep["S"] * prep["n_real"]
             - prep["pads_total"] * float(np.log(32.0))
             - prep["L_sum"]
             + prep["elbo_const"])
    return np.float32(total)
